# revision 1
# baseline (speedup 1.0000x reference)
"""Wilson-Dirac operator on Trainium2, 8 NeuronCores, T-axis domain decomposition.

Lattice 24x24x24x48, complex64 field [X,Y,Z,T,3,4], gauge [4,X,Y,Z,T,3,3].
Sharding: T split into 8 slabs of 6, 1-site halos built host-side (periodic).
Host pre-scales the gauge field by -0.5 (the hopping prefactor) and ships it
twice in direction-specific layouts so every VectorE operand streams with
innermost stride <= 2 elements (strides >= 12B measured 1.25-1.6x slower):

  fh    [X*Y, Z, TS+2, s4, c3, ri2]  field slab, t-halo inline
  ghb   [4, X*Y, Z, TS+1, ri2, b3, a3]  -0.5*U[a,b] at [ri][b][a]  (backward)
  ghf   [4, X*Y, Z, TS+1, ri2, a3, b3]  -0.5*U[a,b] at [ri][a][b]  (forward,
        read transposed as U[b_out,a_out] with steps (3,1))
  outp  [X*Y, Z, TS, s4, c3, ri2]

Compute: partition = (x,y) rows. Half-spinor projection h (j,b,ri), per-site
color products into P (j,th,tu,b,a), in-place b-sum, Re/Im combine into
m (j,a,ri), spin expansion into out accumulator. Shifts: x/y via shifted DMA
row loads, z via in-row AP offsets (periodic wrap split), t via inline halo.
All engine-op APs keep <= 3 free dims (walrus TENSOR3D limit) and the
one-sync-wait-per-instruction walrus limit is handled by splitting waits
onto NoOps at BIR-json level (_split_waits_json)."""

import numpy as np

# ---------------------------------------------------------------- constants
X = Y = Z = 24
T = 48
NCORES = 8
TS = T // NCORES
MASSP4 = 4.5

# h_j = psi[j] + c_j * psi[B_j]; expansion: out[0]+=m[0], out[1]+=m[1],
# out[2] += d0*m[e0], out[3] += d1*m[e1].  Backward: c -> -c, d -> -d.
DIRSPEC = {
    0: dict(B=(3, 2), c=(-1j, -1j), e=(1, 0), d=(+1j, +1j)),
    1: dict(B=(3, 2), c=(-1, +1),   e=(1, 0), d=(+1, -1)),
    2: dict(B=(2, 3), c=(-1j, +1j), e=(0, 1), d=(+1j, -1j)),
    3: dict(B=(2, 3), c=(+1, +1),   e=(0, 1), d=(+1, +1)),
}

_CACHE = {}


def _z_splits(z0, z1, dz, Zn):
    """out z-range [z0,z1) reading input at z+dz (periodic). -> [(oz, n, iz)]"""
    if dz == 0:
        return [(z0, z1 - z0, z0)]
    if dz == -1:
        if z0 == 0:
            out = [(0, 1, Zn - 1)]
            if z1 > 1:
                out.append((1, z1 - 1, 0))
            return out
        return [(z0, z1 - z0, z0 - 1)]
    if dz == +1:
        if z1 == Zn:
            out = []
            if Zn - 1 > z0:
                out.append((z0, Zn - 1 - z0, z0 + 1))
            out.append((Zn - 1, 1, 0))
            return out
        return [(z0, z1 - z0, z0 + 1)]
    raise ValueError(dz)


def _split_waits_json(raw: bytes) -> bytes:
    """Walrus here allows only ONE sync-wait per instruction. Keep the last
    wait on the instruction, hoist the rest onto NoOps inserted immediately
    before it (same engine, semaphores monotonic => exact)."""
    import json
    bj = json.loads(raw)
    nid = 0
    for fn in bj.get("functions", []):
        for bb in fn.get("blocks", []):
            out = []
            changed = False
            for inst in bb.get("instructions", []):
                si = inst.get("sync_info")
                ow = (si or {}).get("on_wait") or []
                if len(ow) > 1:
                    changed = True
                    for w in ow[:-1]:
                        nid += 1
                        out.append({
                            "engine": inst["engine"], "ins": [], "outs": [],
                            "name": f"WSPL-{nid}", "opcode": "NoOp",
                            "sync_info": {"on_update": [], "on_wait": [w]},
                        })
                    si["on_wait"] = [ow[-1]]
                out.append(inst)
            if changed:
                bb["instructions"] = out
    return json.dumps(bj).encode()


def _install_json_wait_fix():
    import concourse.bass as bass
    if getattr(bass.Bass, "_wd_wait_fix", False):
        return
    orig = bass.Bass.to_json_bytes

    def patched(self, *a, **k):
        return _split_waits_json(orig(self, *a, **k))

    bass.Bass.to_json_bytes = patched
    bass.Bass._wd_wait_fix = True


def build_module(Xl, Yl, Zl, TSl, n_zsplit=2, nxc_override=None):
    import concourse.bass as bass
    import concourse.mybir as mybir
    from concourse.ap import AP
    from concourse.mybir import AluOpType
    from concourse.tile import TileContext

    _install_json_wait_fix()

    F32 = mybir.dt.float32
    TH = TSl + 2
    TG = TSl + 1
    XY = Xl * Yl
    NSP = 24
    NSU = 18

    nc = bass.Bass()
    fh = nc.declare_dram_parameter("fh", [XY, Zl, TH, NSP], F32, isOutput=False)
    ghb = nc.declare_dram_parameter("ghb", [4, XY, Zl, TG, NSU], F32, isOutput=False)
    ghf = nc.declare_dram_parameter("ghf", [4, XY, Zl, TG, NSU], F32, isOutput=False)
    outp = nc.declare_dram_parameter("outp", [XY, Zl, TSl, NSP], F32, isOutput=True)

    NXC = nxc_override or max(1, 128 // Yl)
    if n_zsplit > 1 and Zl % n_zsplit == 0:
        zh = Zl // n_zsplit
        zparts = [(i * zh, (i + 1) * zh) for i in range(n_zsplit)]
    else:
        zparts = [(0, Zl)]

    def sap(t, off, dims):
        return AP(t.tensor, t.offset + off, [list(t.ap[0])] + [list(d) for d in dims])

    with TileContext(nc) as tc:
        ctx_pool = tc.tile_pool(name="work", bufs=1)
        pool = ctx_pool.__enter__()
        V = nc.vector
        for x0 in range(0, Xl, NXC):
            nx = min(NXC, Xl - x0)
            R = nx * Yl
            r0 = x0 * Yl

            psi_al = pool.tile([R, Zl * TH * NSP], F32, tag="psi_al", bufs=1)
            out_t = pool.tile([R, Zl * TSl * NSP], F32, tag="out_t", bufs=1)
            # strides (field site-block = (s4, c3, ri2))
            SA = dict(z=TH * NSP, t=NSP, s=6, c=2, ri=1)   # psi_al
            SS = dict(z=TSl * NSP, t=NSP, s=6, c=2, ri=1)  # psi shifted
            SO = dict(z=TSl * NSP, t=NSP, s=6, c=2, ri=1)  # out
            SH = dict(z=TSl * 12, t=12, j=6, b=2, ri=1)    # h
            SU = dict(z=TSl * NSU, t=NSU, ri=9, r3=3, c1=1)  # gauge tiles
            SP_ = dict(z=TSl * 72, t=72, j=36, th=18, tu=9, b=3, a=1)
            SM = dict(z=TSl * 12, t=12, j=6, a=2, ri=1)    # m

            nc.gpsimd.dma_start(out=psi_al[:], in_=fh[r0:r0 + R])

            def load_x(tag, src_tensor, mu, drow, tsl0, tsl1, nreals):
                tl = pool.tile([R, Zl * (tsl1 - tsl0) * nreals], F32, tag=tag,
                               bufs=(8 if tag == "g_al" else 4 if tag == "psi_sh" else 2))
                rs = (r0 + drow) % XY
                if src_tensor is None:
                    src = lambda a, b: fh[a:b, :, tsl0:tsl1]
                else:
                    src = lambda a, b: src_tensor[mu, a:b, :, tsl0:tsl1]
                if rs + R <= XY:
                    nc.gpsimd.dma_start(out=tl[:], in_=src(rs, rs + R))
                else:
                    n1 = XY - rs
                    nc.gpsimd.dma_start(out=tl[0:n1], in_=src(rs, XY))
                    nc.gpsimd.dma_start(out=tl[n1:R], in_=src(0, R - n1))
                return tl

            def load_y(tag, src_tensor, mu, dy, tsl0, tsl1, nreals):
                tl = pool.tile([R, Zl * (tsl1 - tsl0) * nreals], F32, tag=tag,
                               bufs=(8 if tag == "g_al" else 4 if tag == "psi_sh" else 2))
                if src_tensor is None:
                    src = lambda a, b: fh[a:b, :, tsl0:tsl1]
                else:
                    src = lambda a, b: src_tensor[mu, a:b, :, tsl0:tsl1]
                for g in range(nx):
                    xa = x0 + g
                    if dy == +1:
                        nc.sync.dma_start(out=tl[g * Yl:g * Yl + Yl - 1],
                                          in_=src(xa * Yl + 1, xa * Yl + Yl))
                        nc.sync.dma_start(out=tl[g * Yl + Yl - 1:g * Yl + Yl],
                                          in_=src(xa * Yl, xa * Yl + 1))
                    else:
                        nc.sync.dma_start(out=tl[g * Yl + 1:g * Yl + Yl],
                                          in_=src(xa * Yl, xa * Yl + Yl - 1))
                        nc.sync.dma_start(out=tl[g * Yl:g * Yl + 1],
                                          in_=src(xa * Yl + Yl - 1, xa * Yl + Yl))
                return tl

            def load_g(src_tensor, mu, tsl0, tsl1):
                tl = pool.tile([R, Zl * TSl * NSU], F32, tag="g_al", bufs=8)
                nc.gpsimd.dma_start(out=tl[:], in_=src_tensor[mu, r0:r0 + R, :, tsl0:tsl1])
                return tl

            # mass term (ACT)
            nc.scalar.mul(
                sap(out_t, 0, [[SO["z"], Zl], [NSP, TSl], [1, NSP]]),
                sap(psi_al, NSP, [[SA["z"], Zl], [NSP, TSl], [1, NSP]]),
                MASSP4)

            for mu in (2, 3, 0, 1):
                spec = DIRSPEC[mu]
                # gauge tiles: fwd from ghf (transposed-read layout), bwd from ghb
                if mu == 0:
                    g_fwd = load_x("g_al", ghf, 0, -Yl, 1, TSl + 1, NSU)
                    g_bwd = load_g(ghb, 0, 1, TSl + 1)
                    psi_f = load_x("psi_sh", None, None, -Yl, 1, TSl + 1, NSP)
                    psi_b = load_x("psi_sh", None, None, +Yl, 1, TSl + 1, NSP)
                elif mu == 1:
                    g_fwd = load_y("g_al", ghf, 1, -1, 1, TSl + 1, NSU)
                    g_bwd = load_g(ghb, 1, 1, TSl + 1)
                    psi_f = load_y("psi_sh", None, None, -1, 1, TSl + 1, NSP)
                    psi_b = load_y("psi_sh", None, None, +1, 1, TSl + 1, NSP)
                elif mu == 2:
                    g_fwd = load_g(ghf, 2, 1, TSl + 1)
                    g_bwd = load_g(ghb, 2, 1, TSl + 1)
                else:
                    g_fwd = load_g(ghf, 3, 0, TSl)
                    g_bwd = load_g(ghb, 3, 1, TSl + 1)

                for sgn in (+1, -1):
                    fwd = sgn == +1
                    cj = spec["c"] if fwd else tuple(-v for v in spec["c"])
                    dj = spec["d"] if fwd else tuple(-v for v in spec["d"])

                    if mu <= 1:
                        psit, dzp, toffp, SPS = (psi_f if fwd else psi_b), 0, 0, SS
                    elif mu == 2:
                        psit, dzp, toffp, SPS = psi_al, (-1 if fwd else +1), NSP, SA
                    else:
                        psit, dzp, toffp, SPS = psi_al, 0, (0 if fwd else 2 * NSP), SA

                    # --- projection into h (j, b, ri); psi innermost (c,ri)
                    ht = pool.tile([R, Zl * TSl * 12], F32, tag="h", bufs=1)
                    for j in (0, 1):
                        A, B, c = j, spec["B"][j], cj[j]
                        for (oz, nz, iz) in _z_splits(0, Zl, dzp, Zl):
                            hbase = oz * SH["z"] + j * SH["j"]
                            pb = iz * SPS["z"] + toffp
                            zt = [[SPS["z"], nz], [SPS["t"], TSl]]
                            hzt = [[SH["z"], nz], [SH["t"], TSl]]
                            if c.imag == 0.0:
                                op = AluOpType.add if c.real > 0 else AluOpType.subtract
                                V.tensor_tensor(
                                    sap(ht, hbase, hzt + [[1, 6]]),
                                    sap(psit, pb + A * 6, zt + [[1, 6]]),
                                    sap(psit, pb + B * 6, zt + [[1, 6]]),
                                    op)
                            else:
                                sg = 1.0 if c.imag > 0 else -1.0
                                # h_re = psiA_re - sg*psiB_im ; h_im = psiA_im + sg*psiB_re
                                V.tensor_tensor(
                                    sap(ht, hbase, hzt + [[SH["b"], 3]]),
                                    sap(psit, pb + A * 6, zt + [[SPS["c"], 3]]),
                                    sap(psit, pb + B * 6 + 1, zt + [[SPS["c"], 3]]),
                                    AluOpType.subtract if sg > 0 else AluOpType.add)
                                V.tensor_tensor(
                                    sap(ht, hbase + 1, hzt + [[SH["b"], 3]]),
                                    sap(psit, pb + A * 6 + 1, zt + [[SPS["c"], 3]]),
                                    sap(psit, pb + B * 6, zt + [[SPS["c"], 3]]),
                                    AluOpType.add if sg > 0 else AluOpType.subtract)

                    gt = g_fwd if fwd else g_bwd
                    dzu = -1 if (fwd and mu == 2) else 0

                    for (zl0, zl1) in zparts:
                        hz = zl1 - zl0
                        pt = pool.tile([R, hz * TSl * 72], F32, tag="P", bufs=1)
                        mt = pool.tile([R, hz * TSl * 12], F32, tag="m", bufs=1)

                        # --- products: per (j,th,tu): P[zt,(b,a)] = U' * h
                        # out/in0 innermost stride 1, in1 broadcast over a
                        for j in (0, 1):
                            for th in (0, 1):
                                for tu in (0, 1):
                                    for (oz, nz, iz) in _z_splits(zl0, zl1, dzu, Zl):
                                        po = (oz - zl0) * SP_["z"] + j * SP_["j"] + th * SP_["th"] + tu * SP_["tu"]
                                        V.tensor_tensor(
                                            sap(pt, po, [[SP_["t"], nz * TSl], [SP_["b"], 3], [1, 3]]),
                                            sap(gt, iz * SU["z"] + tu * SU["ri"], [[SU["t"], nz * TSl], [3, 3], [1, 3]]),
                                            sap(ht, oz * SH["z"] + j * SH["j"] + th, [[SH["t"], nz * TSl], [SH["b"], 3], [0, 3]]),
                                            AluOpType.mult)

                        # --- b-sum in place: P[b0] += P[b1]; P[b0] += P[b2]
                        bdims = [[SP_["t"], hz * TSl], [SP_["tu"], 8], [1, 3]]
                        V.tensor_tensor(sap(pt, 0, bdims), sap(pt, 0, bdims),
                                        sap(pt, SP_["b"], bdims), AluOpType.add)
                        V.tensor_tensor(sap(pt, 0, bdims), sap(pt, 0, bdims),
                                        sap(pt, 2 * SP_["b"], bdims), AluOpType.add)
                        # --- combine into m (j, a, ri):
                        # m_re = P[rr] +- P[ii]; m_im = P[ir] -+ P[ri]
                        cdims = [[SP_["t"], hz * TSl], [SP_["j"], 2], [1, 3]]
                        mdims = [[SM["t"], hz * TSl], [SM["j"], 2], [SM["a"], 3]]
                        RR, II = 0, SP_["th"] + SP_["tu"]
                        IR, RI = SP_["th"], SP_["tu"]
                        V.tensor_tensor(sap(mt, 0, mdims), sap(pt, RR, cdims), sap(pt, II, cdims),
                                        AluOpType.add if fwd else AluOpType.subtract)
                        V.tensor_tensor(sap(mt, 1, mdims), sap(pt, IR, cdims), sap(pt, RI, cdims),
                                        AluOpType.subtract if fwd else AluOpType.add)

                        # --- expansion into out_t (s,c,ri layout; (c,ri)=[1,6])
                        ob = zl0 * SO["z"]
                        ozt = [[NSP, hz * TSl]]
                        mzt = [[SM["t"], hz * TSl]]
                        for s in (0, 1):
                            os_ = sap(out_t, ob + s * SO["s"], ozt + [[1, 6]])
                            V.tensor_tensor(os_, os_, sap(mt, s * SM["j"], mzt + [[1, 6]]),
                                            AluOpType.add)
                        for si_, (ei, dv) in enumerate(zip(spec["e"], dj)):
                            sb = ob + (2 + si_) * SO["s"]
                            if dv.imag == 0.0:
                                op = AluOpType.add if dv.real > 0 else AluOpType.subtract
                                os_ = sap(out_t, sb, ozt + [[1, 6]])
                                V.tensor_tensor(os_, os_, sap(mt, ei * SM["j"], mzt + [[1, 6]]), op)
                            else:
                                sg = 1.0 if dv.imag > 0 else -1.0
                                # out_re += -sg*m_im ; out_im += sg*m_re
                                ore = sap(out_t, sb, ozt + [[SO["c"], 3]])
                                V.tensor_tensor(ore, ore,
                                                sap(mt, ei * SM["j"] + 1, mzt + [[SM["a"], 3]]),
                                                AluOpType.subtract if sg > 0 else AluOpType.add)
                                oim = sap(out_t, sb + 1, ozt + [[SO["c"], 3]])
                                V.tensor_tensor(oim, oim,
                                                sap(mt, ei * SM["j"], mzt + [[SM["a"], 3]]),
                                                AluOpType.add if sg > 0 else AluOpType.subtract)

            nc.gpsimd.dma_start(out=outp[r0:r0 + R], in_=out_t[:])
        ctx_pool.__exit__(None, None, None)
    return nc


# ---------------------------------------------------------------- host side
def _prep_core_inputs(fv, gv, t0, Xl, Yl, Zl, Tl, TSl):
    """fv: [X,Y,Z,T,3,4,2] f32 view (c,s,ri). gv: [4,X,Y,Z,T,3,3,2] (a,b,ri).
    Returns fh [XY,Z,TH,(s,c,ri)], ghb [...,(ri,b,a)], ghf [...,(ri,a,b)],
    gauge pre-scaled by -0.5."""
    idx = [(t0 - 1) % Tl] + [(t0 + i) % Tl for i in range(TSl)] + [(t0 + TSl) % Tl]
    f = fv[:, :, :, idx]                       # [X,Y,Z,TH,c,s,ri]
    f = f.transpose(0, 1, 2, 3, 5, 4, 6)       # -> (s,c,ri)
    fhn = np.ascontiguousarray(f).reshape(Xl * Yl, Zl, TSl + 2, 24)
    idg = [(t0 - 1 + i) % Tl for i in range(TSl + 1)]
    g = gv[:, :, :, :, idg]                    # [4,X,Y,Z,TG,a,b,ri]
    ghfn = np.ascontiguousarray(g.transpose(0, 1, 2, 3, 4, 7, 5, 6))  # (ri,a,b)
    ghbn = np.ascontiguousarray(g.transpose(0, 1, 2, 3, 4, 7, 6, 5))  # (ri,b,a)
    ghfn *= -0.5
    ghbn *= -0.5
    return (fhn, ghfn.reshape(4, Xl * Yl, Zl, TSl + 1, 18),
            ghbn.reshape(4, Xl * Yl, Zl, TSl + 1, 18))


def _out_to_complex(o, Xl, Yl, Zl, TSl):
    o = o.reshape(Xl, Yl, Zl, TSl, 4, 3, 2)    # (s,c,ri)
    o = o.transpose(0, 1, 2, 3, 5, 4, 6)       # -> (c,s,ri)
    return o[..., 0] + 1j * o[..., 1]


def kernel(field, gauge_field):
    from concourse.bass_utils import run_bass_kernel_spmd

    key = "full"
    if key not in _CACHE:
        _CACHE[key] = build_module(X, Y, Z, TS)
    nc = _CACHE[key]

    fv = np.ascontiguousarray(field).view(np.float32).reshape(X, Y, Z, T, 3, 4, 2)
    gv = np.ascontiguousarray(gauge_field).view(np.float32).reshape(4, X, Y, Z, T, 3, 3, 2)

    in_maps = []
    for k in range(NCORES):
        fhn, ghfn, ghbn = _prep_core_inputs(fv, gv, k * TS, X, Y, Z, T, TS)
        in_maps.append({"fh": fhn, "ghf": ghfn, "ghb": ghbn})

    res = run_bass_kernel_spmd(nc, in_maps, list(range(NCORES))).results

    out = np.empty((X, Y, Z, T, 3, 4), np.complex64)
    for k in range(NCORES):
        out[:, :, :, k * TS:(k + 1) * TS] = _out_to_complex(
            res[k]["outp"], X, Y, Z, TS)
    return out



# revision 2
# speedup vs baseline: 1.6569x; 1.6569x over previous
"""Wilson-Dirac operator on Trainium2, 8 NeuronCores, T-axis domain decomposition.

v2: all-f16 compute (DVE 2x packed mode, measured 1.87 el/ns vs 0.93 f32),
host pre-rolled/pre-transposed operand arrays so every DMA is a contiguous
row-range read, and a product layout P'[j,g,A,B] whose broadcast rides the
OUTER free dim so all hot ops keep innermost stride 1.

Host arrays per core (f16, gauge pre-scaled by -0.5, fwd links pre-rolled):
  psi_h [XY, Z+2, TS+2, 24]  psi slab, z+t halos inline, comps (s,p,c)
  fi4   [4, XY, Z, TS, 24]   interior psi pre-rolled (x+1, x-1, y+1, y-1)
  WF/WB [4, XY, Z, TS, 18]   hopping matrices, comps (p, A, B)
  outp  [XY, Z, TS, 24]      output, comps (s,p,c)

Blocks: 4 x (128 rows, z 0..24) + 1 x (64 rows, z split in half across the
128 partitions) = 4.5 block-passes instead of 5.

Per (mu, sgn): proj h[j,p,b] -> products P'[j,g,A,B] (W[p,A,B] stride-1,
h broadcast on outer A) -> bsum over B (S[j,g,A], gpsimd add1) -> combine
m[j,p,a] -> expand into out accumulator. Mass term on ACT engine.
"""

import numpy as np

# ---------------------------------------------------------------- constants
X = Y = Z = 24
T = 48
NCORES = 8
TS = T // NCORES
TH = TS + 2
XY = X * Y
MASSP4 = 4.5

DIRSPEC = {
    0: dict(B=(3, 2), c=(-1j, -1j), e=(1, 0), d=(+1j, +1j)),
    1: dict(B=(3, 2), c=(-1, +1),   e=(1, 0), d=(+1, -1)),
    2: dict(B=(2, 3), c=(-1j, +1j), e=(0, 1), d=(+1j, -1j)),
    3: dict(B=(2, 3), c=(+1, +1),   e=(0, 1), d=(+1, +1)),
}

_CACHE = {}


def _split_waits_json(raw: bytes) -> bytes:
    """Walrus allows only ONE sync-wait per instruction: hoist extras onto
    NoOps inserted immediately before (same engine; sems monotonic => exact)."""
    import json
    bj = json.loads(raw)
    nid = 0
    for fn in bj.get("functions", []):
        for bb in fn.get("blocks", []):
            out = []
            changed = False
            for inst in bb.get("instructions", []):
                si = inst.get("sync_info")
                ow = (si or {}).get("on_wait") or []
                if len(ow) > 1:
                    changed = True
                    for w in ow[:-1]:
                        nid += 1
                        out.append({
                            "engine": inst["engine"], "ins": [], "outs": [],
                            "name": f"WSPL-{nid}", "opcode": "NoOp",
                            "sync_info": {"on_update": [], "on_wait": [w]},
                        })
                    si["on_wait"] = [ow[-1]]
                out.append(inst)
            if changed:
                bb["instructions"] = out
    return json.dumps(bj).encode()


def _install_json_wait_fix():
    import concourse.bass as bass
    if getattr(bass.Bass, "_wd_wait_fix", False):
        return
    orig = bass.Bass.to_json_bytes

    def patched(self, *a, **k):
        return _split_waits_json(orig(self, *a, **k))

    bass.Bass.to_json_bytes = patched
    bass.Bass._wd_wait_fix = True


def build_module(pool_bsum=True):
    import concourse.bass as bass
    import concourse.mybir as mybir
    from concourse.ap import AP
    from concourse.mybir import AluOpType
    from concourse.tile import TileContext

    _install_json_wait_fix()
    F16 = mybir.dt.float16

    nc = bass.Bass()
    psi_h = nc.declare_dram_parameter("psi_h", [XY, (Z + 2) * TH * 24], F16, isOutput=False)
    fi4 = nc.declare_dram_parameter("fi4", [4, XY, Z * TS * 24], F16, isOutput=False)
    WFp = nc.declare_dram_parameter("WF", [4, XY, Z * TS * 18], F16, isOutput=False)
    WBp = nc.declare_dram_parameter("WB", [4, XY, Z * TS * 18], F16, isOutput=False)
    outp = nc.declare_dram_parameter("outp", [XY, Z * TS * 24], F16, isOutput=True)

    # blocks: (r0, R, [(z0, zh, part0), ...]) — parts map z-ranges to
    # partition sub-ranges; compute dims use zh of parts[0] (all equal).
    blocks = [(0, 128, [(0, 24, 0)]), (128, 128, [(0, 24, 0)]),
              (256, 128, [(0, 24, 0)]), (384, 128, [(0, 24, 0)]),
              (512, 64, [(0, 12, 0), (12, 12, 64)])]

    def sap(t, off, dims):
        return AP(t.tensor, t.offset + off, [list(t.ap[0])] + [list(d) for d in dims])

    with TileContext(nc) as tc:
        ctx_pool = tc.tile_pool(name="work", bufs=1)
        pool = ctx_pool.__enter__()
        V = nc.vector
        G = nc.gpsimd
        D = nc.sync
        for (r0, R, parts) in blocks:
            zh = parts[0][1]
            npart = R * len(parts)  # partitions in use
            ZT6 = zh * TS

            psi_al = pool.tile([npart, (zh + 2) * TH * 24], F16, tag="psi_al", bufs=2)
            out_t = pool.tile([npart, ZT6 * 24], F16, tag="out_t", bufs=2)
            for (z0, _, p0) in parts:
                D.dma_start(out=psi_al[p0:p0 + R],
                            in_=psi_h[r0:r0 + R, z0 * TH * 24:(z0 + zh + 2) * TH * 24])

            def load(tag, src, bufs, nc_):
                tl = pool.tile([npart, ZT6 * nc_], F16, tag=tag, bufs=bufs)
                for (z0, _, p0) in parts:
                    D.dma_start(out=tl[p0:p0 + R],
                                in_=src[r0:r0 + R, z0 * TS * nc_:(z0 + zh) * TS * nc_])
                return tl

            # mass term on ACT: out = 4.5 * psi interior
            nc.scalar.mul(
                sap(out_t, 0, [[144, zh], [24, 6], [1, 24]]),
                sap(psi_al, TH * 24 + 24, [[TH * 24, zh], [24, 6], [1, 24]]),
                MASSP4)

            for mu in range(4):
                wf_t = load("w", WFp[mu], 6, 18)
                wb_t = load("w", WBp[mu], 6, 18)
                if mu <= 1:
                    pf_t = load("fi", fi4[2 * mu], 8, 24)
                    pb_t = load("fi", fi4[2 * mu + 1], 8, 24)
                spec = DIRSPEC[mu]

                for sgn in (+1, -1):
                    fwd = sgn > 0
                    cj = spec["c"] if fwd else tuple(-v for v in spec["c"])
                    dj = spec["d"] if fwd else tuple(-v for v in spec["d"])
                    wt = wf_t if fwd else wb_t

                    # psi source: (tile, base, zdims) — zdims is the (z,t)
                    # prefix of the free dims ([[stride, n], ...]).
                    if mu <= 1:
                        ps, pbase, pz = (pf_t if fwd else pb_t), 0, [[24, ZT6]]
                    elif mu == 2:
                        pbase = (0 if fwd else 2 * TH * 24) + 24
                        ps, pz = psi_al, [[TH * 24, zh], [24, 6]]
                    else:
                        pbase = TH * 24 + (0 if fwd else 48)
                        ps, pz = psi_al, [[TH * 24, zh], [24, 6]]

                    hz = [[12, ZT6]] if len(pz) == 1 else [[72, zh], [12, 6]]

                    # --- proj h[j,p,b] = psi[A] + c*psi[B]
                    ht = pool.tile([npart, ZT6 * 12], F16, tag="h", bufs=2)
                    for j in (0, 1):
                        A, B, c = j, spec["B"][j], cj[j]
                        if c.imag == 0.0:
                            op = AluOpType.add if c.real > 0 else AluOpType.subtract
                            V.tensor_tensor(
                                sap(ht, j * 6, hz + [[1, 6]]),
                                sap(ps, pbase + A * 6, pz + [[1, 6]]),
                                sap(ps, pbase + B * 6, pz + [[1, 6]]), op)
                        else:
                            sg = c.imag > 0
                            V.tensor_tensor(
                                sap(ht, j * 6, hz + [[1, 3]]),
                                sap(ps, pbase + A * 6, pz + [[1, 3]]),
                                sap(ps, pbase + B * 6 + 3, pz + [[1, 3]]),
                                AluOpType.subtract if sg else AluOpType.add)
                            V.tensor_tensor(
                                sap(ht, j * 6 + 3, hz + [[1, 3]]),
                                sap(ps, pbase + A * 6 + 3, pz + [[1, 3]]),
                                sap(ps, pbase + B * 6, pz + [[1, 3]]),
                                AluOpType.add if sg else AluOpType.subtract)

                    # --- products P'[j,g,A,B] = W[p,A,B] * h[j,p,B]
                    # g: (Wre*hre, Wim*him, Wim*hre, Wre*him)
                    pt = pool.tile([npart, ZT6 * 72], F16, tag="P", bufs=2)
                    for j in (0, 1):
                        for g, (wp, hp) in enumerate(((0, 0), (1, 1), (1, 0), (0, 1))):
                            V.tensor_tensor(
                                sap(pt, j * 36 + g * 9, [[72, ZT6], [3, 3], [1, 3]]),
                                sap(wt, wp * 9, [[18, ZT6], [3, 3], [1, 3]]),
                                sap(ht, j * 6 + hp * 3, [[12, ZT6], [0, 3], [1, 3]]),
                                AluOpType.mult)

                    # --- bsum over B: S[j,g,A] (inner stride 3 on P => 1x)
                    st = pool.tile([npart, ZT6 * 24], F16, tag="S", bufs=2)
                    E1 = G if pool_bsum else V
                    E1.tensor_tensor(sap(st, 0, [[24, ZT6], [1, 24]]),
                                     sap(pt, 0, [[72, ZT6], [3, 24]]),
                                     sap(pt, 1, [[72, ZT6], [3, 24]]), AluOpType.add)
                    V.tensor_tensor(sap(st, 0, [[24, ZT6], [1, 24]]),
                                    sap(st, 0, [[24, ZT6], [1, 24]]),
                                    sap(pt, 2, [[72, ZT6], [3, 24]]), AluOpType.add)

                    # --- combine m[j,p,a]: m_re = S[g0]-S[g1]; m_im = S[g2]+S[g3]
                    mt = pool.tile([npart, ZT6 * 12], F16, tag="m", bufs=2)
                    V.tensor_tensor(sap(mt, 0, [[12, ZT6], [6, 2], [1, 3]]),
                                    sap(st, 0, [[24, ZT6], [12, 2], [1, 3]]),
                                    sap(st, 3, [[24, ZT6], [12, 2], [1, 3]]),
                                    AluOpType.subtract)
                    V.tensor_tensor(sap(mt, 3, [[12, ZT6], [6, 2], [1, 3]]),
                                    sap(st, 6, [[24, ZT6], [12, 2], [1, 3]]),
                                    sap(st, 9, [[24, ZT6], [12, 2], [1, 3]]),
                                    AluOpType.add)

                    # --- expand into out_t
                    o01 = sap(out_t, 0, [[24, ZT6], [6, 2], [1, 6]])
                    V.tensor_tensor(o01, o01, sap(mt, 0, [[12, ZT6], [6, 2], [1, 6]]),
                                    AluOpType.add)
                    e0, e1 = spec["e"]
                    d0, d1 = dj
                    if d0.imag == 0.0:
                        if d0.real == d1.real and e0 == 0:
                            o23 = sap(out_t, 12, [[24, ZT6], [6, 2], [1, 6]])
                            V.tensor_tensor(o23, o23,
                                            sap(mt, 0, [[12, ZT6], [6, 2], [1, 6]]),
                                            AluOpType.add if d0.real > 0 else AluOpType.subtract)
                        else:
                            for si, (e, dv) in enumerate(zip(spec["e"], dj)):
                                os_ = sap(out_t, 12 + 6 * si, [[24, ZT6], [1, 6]])
                                V.tensor_tensor(os_, os_, sap(mt, e * 6, [[12, ZT6], [1, 6]]),
                                                AluOpType.add if dv.real > 0 else AluOpType.subtract)
                    else:
                        for si, (e, dv) in enumerate(zip(spec["e"], dj)):
                            sg = dv.imag > 0
                            ore = sap(out_t, 12 + 6 * si, [[24, ZT6], [1, 3]])
                            V.tensor_tensor(ore, ore, sap(mt, e * 6 + 3, [[12, ZT6], [1, 3]]),
                                            AluOpType.subtract if sg else AluOpType.add)
                            oim = sap(out_t, 12 + 6 * si + 3, [[24, ZT6], [1, 3]])
                            V.tensor_tensor(oim, oim, sap(mt, e * 6, [[12, ZT6], [1, 3]]),
                                            AluOpType.add if sg else AluOpType.subtract)

            for (z0, _, p0) in parts:
                D.dma_start(out=outp[r0:r0 + R, z0 * TS * 24:(z0 + zh) * TS * 24],
                            in_=out_t[p0:p0 + R])
        ctx_pool.__exit__(None, None, None)
    return nc


# ---------------------------------------------------------------- host side
def prep_core_inputs(field, gauge, t0):
    """field [X,Y,Z,T,3,4] c64, gauge [4,X,Y,Z,T,3,3] c64 -> f16 arrays."""
    tsl = [(t0 + i) % T for i in range(TS)]
    th_idx = [(t0 - 1) % T] + tsl + [(t0 + TS) % T]
    f = field[:, :, :, th_idx]
    fr = np.stack([f.real, f.imag], axis=-1)            # [X,Y,Z,TH,c,s,p]
    fspc = fr.transpose(0, 1, 2, 3, 5, 6, 4)            # [X,Y,Z,TH,s,p,c]
    zhal = np.concatenate([fspc[:, :, -1:], fspc, fspc[:, :, :1]], axis=2)
    psi_h = np.ascontiguousarray(zhal).reshape(XY, (Z + 2) * TH * 24).astype(np.float16)

    fin = fspc[:, :, :, 1:TS + 1]
    rolls = [np.roll(fin, +1, 0), np.roll(fin, -1, 0),
             np.roll(fin, +1, 1), np.roll(fin, -1, 1)]
    fi4 = np.stack([np.ascontiguousarray(r).reshape(XY, Z * TS * 24) for r in rolls]
                   ).astype(np.float16)

    WF = np.empty((4, XY, Z * TS * 18), np.float16)
    WB = np.empty((4, XY, Z * TS * 18), np.float16)
    for mu in range(4):
        Ub = gauge[mu][:, :, :, tsl]
        vb = np.stack([Ub.real, Ub.imag], axis=-3) * np.float32(-0.5)
        WB[mu] = np.ascontiguousarray(vb).reshape(XY, Z * TS * 18).astype(np.float16)
        if mu == 3:
            tf = [(t0 - 1 + i) % T for i in range(TS)]
            Uf = gauge[mu][:, :, :, tf]
        else:
            Uf = np.roll(gauge[mu], +1, axis=mu)[:, :, :, tsl]
        Vf = np.conjugate(np.swapaxes(Uf, -1, -2))
        vf = np.stack([Vf.real, Vf.imag], axis=-3) * np.float32(-0.5)
        WF[mu] = np.ascontiguousarray(vf).reshape(XY, Z * TS * 18).astype(np.float16)
    return {"psi_h": psi_h, "fi4": fi4, "WF": WF, "WB": WB}


def prep_in_maps(field, gauge):
    return [prep_core_inputs(field, gauge, k * TS) for k in range(NCORES)]


def assemble_output(res):
    out = np.empty((X, Y, Z, T, 3, 4), np.complex64)
    for k in range(NCORES):
        o = res[k]["outp"].reshape(X, Y, Z, TS, 4, 2, 3).astype(np.float32)
        oc = (o[..., 0, :] + 1j * o[..., 1, :]).transpose(0, 1, 2, 3, 5, 4)
        out[:, :, :, k * TS:(k + 1) * TS] = oc
    return out


def kernel(field, gauge_field):
    from concourse.bass_utils import run_bass_kernel_spmd

    if "v2" not in _CACHE:
        _CACHE["v2"] = build_module()
    nc = _CACHE["v2"]
    in_maps = prep_in_maps(np.asarray(field), np.asarray(gauge_field))
    res = run_bass_kernel_spmd(nc, in_maps, list(range(NCORES))).results
    return assemble_output(res)


# revision 5
# speedup vs baseline: 1.6590x; 1.0013x over previous
"""Wilson-Dirac operator on Trainium2, 8 NeuronCores, T-axis domain decomposition.

v2: all-f16 compute (DVE 2x packed mode, measured 1.87 el/ns vs 0.93 f32),
host pre-rolled/pre-transposed operand arrays so every DMA is a contiguous
row-range read, and a product layout P'[j,g,A,B] whose broadcast rides the
OUTER free dim so all hot ops keep innermost stride 1.

Host arrays per core (f16, gauge pre-scaled by -0.5, fwd links pre-rolled):
  psi_h [XY, Z+2, TS+2, 24]  psi slab, z+t halos inline, comps (s,p,c)
  fi4   [4, XY, Z, TS, 24]   interior psi pre-rolled (x+1, x-1, y+1, y-1)
  WF/WB [4, XY, Z, TS, 18]   hopping matrices, comps (p, A, B)
  outp  [XY, Z, TS, 24]      output, comps (s,p,c)

Blocks: 4 x (128 rows, z 0..24) + 1 x (64 rows, z split in half across the
128 partitions) = 4.5 block-passes instead of 5.

Per (mu, sgn): proj h[j,p,b] -> products P'[j,g,A,B] (W[p,A,B] stride-1,
h broadcast on outer A) -> bsum over B (S[j,g,A], gpsimd add1) -> combine
m[j,p,a] -> expand into out accumulator. Mass term on ACT engine.
"""

import numpy as np

# ---------------------------------------------------------------- constants
X = Y = Z = 24
T = 48
NCORES = 8
TS = T // NCORES
TH = TS + 2
XY = X * Y
MASSP4 = 4.5

DIRSPEC = {
    0: dict(B=(3, 2), c=(-1j, -1j), e=(1, 0), d=(+1j, +1j)),
    1: dict(B=(3, 2), c=(-1, +1),   e=(1, 0), d=(+1, -1)),
    2: dict(B=(2, 3), c=(-1j, +1j), e=(0, 1), d=(+1j, -1j)),
    3: dict(B=(2, 3), c=(+1, +1),   e=(0, 1), d=(+1, +1)),
}

_CACHE = {}


def _split_waits_json(raw: bytes) -> bytes:
    """Walrus allows only ONE sync-wait per instruction: hoist extras onto
    NoOps inserted immediately before (same engine; sems monotonic => exact)."""
    import json
    bj = json.loads(raw)
    nid = 0
    for fn in bj.get("functions", []):
        for bb in fn.get("blocks", []):
            out = []
            changed = False
            for inst in bb.get("instructions", []):
                si = inst.get("sync_info")
                ow = (si or {}).get("on_wait") or []
                if len(ow) > 1:
                    changed = True
                    for w in ow[:-1]:
                        nid += 1
                        out.append({
                            "engine": inst["engine"], "ins": [], "outs": [],
                            "name": f"WSPL-{nid}", "opcode": "NoOp",
                            "sync_info": {"on_update": [], "on_wait": [w]},
                        })
                    si["on_wait"] = [ow[-1]]
                out.append(inst)
            if changed:
                bb["instructions"] = out
    return json.dumps(bj).encode()


def _install_json_wait_fix():
    import concourse.bass as bass
    if getattr(bass.Bass, "_wd_wait_fix", False):
        return
    orig = bass.Bass.to_json_bytes

    def patched(self, *a, **k):
        return _split_waits_json(orig(self, *a, **k))

    bass.Bass.to_json_bytes = patched
    bass.Bass._wd_wait_fix = True


def build_module(pool_bsum=True):
    import concourse.bass as bass
    import concourse.mybir as mybir
    from concourse.ap import AP
    from concourse.mybir import AluOpType
    from concourse.tile import TileContext

    _install_json_wait_fix()
    F16 = mybir.dt.float16

    nc = bass.Bass()
    psi_h = nc.declare_dram_parameter("psi_h", [XY, (Z + 2) * TH * 24], F16, isOutput=False)
    fi4 = nc.declare_dram_parameter("fi4", [4, XY, Z * TS * 24], F16, isOutput=False)
    WFp = nc.declare_dram_parameter("WF", [4, XY, Z * TS * 18], F16, isOutput=False)
    WBp = nc.declare_dram_parameter("WB", [4, XY, Z * TS * 18], F16, isOutput=False)
    outp = nc.declare_dram_parameter("outp", [XY, Z * TS * 24], F16, isOutput=True)

    # blocks: (r0, R, [(z0, zh, part0), ...]) — parts map z-ranges to
    # partition sub-ranges; compute dims use zh of parts[0] (all equal).
    blocks = [(0, 128, [(0, 24, 0)]), (128, 128, [(0, 24, 0)]),
              (256, 128, [(0, 24, 0)]), (384, 128, [(0, 24, 0)]),
              (512, 64, [(0, 12, 0), (12, 12, 64)])]

    def sap(t, off, dims):
        return AP(t.tensor, t.offset + off, [list(t.ap[0])] + [list(d) for d in dims])

    with TileContext(nc) as tc:
        ctx_pool = tc.tile_pool(name="work", bufs=1)
        pool = ctx_pool.__enter__()
        V = nc.vector
        G = nc.gpsimd
        D = nc.sync
        for (r0, R, parts) in blocks:
            zh = parts[0][1]
            npart = R * len(parts)  # partitions in use
            ZT6 = zh * TS

            psi_al = pool.tile([npart, (zh + 2) * TH * 24], F16, tag="psi_al", bufs=2)
            out_t = pool.tile([npart, ZT6 * 24], F16, tag="out_t", bufs=2)
            for (z0, _, p0) in parts:
                D.dma_start(out=psi_al[p0:p0 + R],
                            in_=psi_h[r0:r0 + R, z0 * TH * 24:(z0 + zh + 2) * TH * 24])

            def load(tag, src, bufs, nc_):
                tl = pool.tile([npart, ZT6 * nc_], F16, tag=tag, bufs=bufs)
                for (z0, _, p0) in parts:
                    D.dma_start(out=tl[p0:p0 + R],
                                in_=src[r0:r0 + R, z0 * TS * nc_:(z0 + zh) * TS * nc_])
                return tl

            # mass term on ACT: out = 4.5 * psi interior
            nc.scalar.mul(
                sap(out_t, 0, [[144, zh], [24, 6], [1, 24]]),
                sap(psi_al, TH * 24 + 24, [[TH * 24, zh], [24, 6], [1, 24]]),
                MASSP4)

            tail_q = []

            def run_tail():
                if not tail_q:
                    return
                pt, st, mt, spec, dj = tail_q.pop()
                # --- bsum part 2 (DVE; add1 ran on gpsimd one term ago)
                V.tensor_tensor(sap(st, 0, [[24, ZT6], [1, 24]]),
                                sap(st, 0, [[24, ZT6], [1, 24]]),
                                sap(pt, 2, [[72, ZT6], [3, 24]]), AluOpType.add)
                # --- combine m[j,p,a]: m_re = S[g0]-S[g1]; m_im = S[g2]+S[g3]
                V.tensor_tensor(sap(mt, 0, [[12, ZT6], [6, 2], [1, 3]]),
                                sap(st, 0, [[24, ZT6], [12, 2], [1, 3]]),
                                sap(st, 3, [[24, ZT6], [12, 2], [1, 3]]),
                                AluOpType.subtract)
                V.tensor_tensor(sap(mt, 3, [[12, ZT6], [6, 2], [1, 3]]),
                                sap(st, 6, [[24, ZT6], [12, 2], [1, 3]]),
                                sap(st, 9, [[24, ZT6], [12, 2], [1, 3]]),
                                AluOpType.add)
                # --- expand into out_t
                o01 = sap(out_t, 0, [[24, ZT6], [6, 2], [1, 6]])
                V.tensor_tensor(o01, o01, sap(mt, 0, [[12, ZT6], [6, 2], [1, 6]]),
                                AluOpType.add)
                d0, d1 = dj
                if d0.imag == 0.0:
                    if d0.real == d1.real and spec["e"][0] == 0:
                        o23 = sap(out_t, 12, [[24, ZT6], [6, 2], [1, 6]])
                        V.tensor_tensor(o23, o23,
                                        sap(mt, 0, [[12, ZT6], [6, 2], [1, 6]]),
                                        AluOpType.add if d0.real > 0 else AluOpType.subtract)
                    else:
                        for si, (e, dv) in enumerate(zip(spec["e"], dj)):
                            os_ = sap(out_t, 12 + 6 * si, [[24, ZT6], [1, 6]])
                            V.tensor_tensor(os_, os_, sap(mt, e * 6, [[12, ZT6], [1, 6]]),
                                            AluOpType.add if dv.real > 0 else AluOpType.subtract)
                else:
                    for si, (e, dv) in enumerate(zip(spec["e"], dj)):
                        sg = dv.imag > 0
                        ore = sap(out_t, 12 + 6 * si, [[24, ZT6], [1, 3]])
                        V.tensor_tensor(ore, ore, sap(mt, e * 6 + 3, [[12, ZT6], [1, 3]]),
                                        AluOpType.subtract if sg else AluOpType.add)
                        oim = sap(out_t, 12 + 6 * si + 3, [[24, ZT6], [1, 3]])
                        V.tensor_tensor(oim, oim, sap(mt, e * 6, [[12, ZT6], [1, 3]]),
                                        AluOpType.add if sg else AluOpType.subtract)

            for mu in range(4):
                wf_t = load("w", WFp[mu], 6, 18)
                wb_t = load("w", WBp[mu], 6, 18)
                if mu <= 1:
                    pf_t = load("fi", fi4[2 * mu], 8, 24)
                    pb_t = load("fi", fi4[2 * mu + 1], 8, 24)
                spec = DIRSPEC[mu]

                for sgn in (+1, -1):
                    fwd = sgn > 0
                    cj = spec["c"] if fwd else tuple(-v for v in spec["c"])
                    dj = spec["d"] if fwd else tuple(-v for v in spec["d"])
                    wt = wf_t if fwd else wb_t

                    # psi source: (tile, base, zdims) — zdims is the (z,t)
                    # prefix of the free dims ([[stride, n], ...]).
                    if mu <= 1:
                        ps, pbase, pz = (pf_t if fwd else pb_t), 0, [[24, ZT6]]
                    elif mu == 2:
                        pbase = (0 if fwd else 2 * TH * 24) + 24
                        ps, pz = psi_al, [[TH * 24, zh], [24, 6]]
                    else:
                        pbase = TH * 24 + (0 if fwd else 48)
                        ps, pz = psi_al, [[TH * 24, zh], [24, 6]]

                    hz = [[12, ZT6]] if len(pz) == 1 else [[72, zh], [12, 6]]

                    # --- proj h[j,p,b] = psi[A] + c*psi[B]
                    ht = pool.tile([npart, ZT6 * 12], F16, tag="h", bufs=2)
                    for j in (0, 1):
                        A, B, c = j, spec["B"][j], cj[j]
                        if c.imag == 0.0:
                            op = AluOpType.add if c.real > 0 else AluOpType.subtract
                            V.tensor_tensor(
                                sap(ht, j * 6, hz + [[1, 6]]),
                                sap(ps, pbase + A * 6, pz + [[1, 6]]),
                                sap(ps, pbase + B * 6, pz + [[1, 6]]), op)
                        else:
                            sg = c.imag > 0
                            V.tensor_tensor(
                                sap(ht, j * 6, hz + [[1, 3]]),
                                sap(ps, pbase + A * 6, pz + [[1, 3]]),
                                sap(ps, pbase + B * 6 + 3, pz + [[1, 3]]),
                                AluOpType.subtract if sg else AluOpType.add)
                            V.tensor_tensor(
                                sap(ht, j * 6 + 3, hz + [[1, 3]]),
                                sap(ps, pbase + A * 6 + 3, pz + [[1, 3]]),
                                sap(ps, pbase + B * 6, pz + [[1, 3]]),
                                AluOpType.add if sg else AluOpType.subtract)

                    # --- products P'[j,g,A,B] = W[p,A,B] * h[j,p,B]
                    # g: (Wre*hre, Wim*him, Wim*hre, Wre*him)
                    pt = pool.tile([npart, ZT6 * 72], F16, tag="P", bufs=2)
                    for j in (0, 1):
                        for g, (wp, hp) in enumerate(((0, 0), (1, 1), (1, 0), (0, 1))):
                            V.tensor_tensor(
                                sap(pt, j * 36 + g * 9, [[72, ZT6], [3, 3], [1, 3]]),
                                sap(wt, wp * 9, [[18, ZT6], [3, 3], [1, 3]]),
                                sap(ht, j * 6 + hp * 3, [[12, ZT6], [0, 3], [1, 3]]),
                                AluOpType.mult)

                    # --- bsum part 1 on gpsimd: S = P[B0] + P[B1]
                    st = pool.tile([npart, ZT6 * 24], F16, tag="S", bufs=3)
                    E1 = G if pool_bsum else V
                    E1.tensor_tensor(sap(st, 0, [[24, ZT6], [1, 24]]),
                                     sap(pt, 0, [[72, ZT6], [3, 24]]),
                                     sap(pt, 1, [[72, ZT6], [3, 24]]), AluOpType.add)
                    mt = pool.tile([npart, ZT6 * 12], F16, tag="m", bufs=3)

                    # deferred tail (prev term) runs now; gpsimd add1 of THIS
                    # term overlaps with it.
                    run_tail()
                    tail_q.append((pt, st, mt, spec, dj))

            run_tail()
            for (z0, _, p0) in parts:
                nc.scalar.dma_start(out=outp[r0:r0 + R, z0 * TS * 24:(z0 + zh) * TS * 24],
                                    in_=out_t[p0:p0 + R])
        ctx_pool.__exit__(None, None, None)
    return nc


# ---------------------------------------------------------------- host side
def prep_core_inputs(field, gauge, t0):
    """field [X,Y,Z,T,3,4] c64, gauge [4,X,Y,Z,T,3,3] c64 -> f16 arrays."""
    tsl = [(t0 + i) % T for i in range(TS)]
    th_idx = [(t0 - 1) % T] + tsl + [(t0 + TS) % T]
    f = field[:, :, :, th_idx]
    fr = np.stack([f.real, f.imag], axis=-1)            # [X,Y,Z,TH,c,s,p]
    fspc = fr.transpose(0, 1, 2, 3, 5, 6, 4)            # [X,Y,Z,TH,s,p,c]
    zhal = np.concatenate([fspc[:, :, -1:], fspc, fspc[:, :, :1]], axis=2)
    psi_h = np.ascontiguousarray(zhal).reshape(XY, (Z + 2) * TH * 24).astype(np.float16)

    fin = fspc[:, :, :, 1:TS + 1]
    rolls = [np.roll(fin, +1, 0), np.roll(fin, -1, 0),
             np.roll(fin, +1, 1), np.roll(fin, -1, 1)]
    fi4 = np.stack([np.ascontiguousarray(r).reshape(XY, Z * TS * 24) for r in rolls]
                   ).astype(np.float16)

    WF = np.empty((4, XY, Z * TS * 18), np.float16)
    WB = np.empty((4, XY, Z * TS * 18), np.float16)
    for mu in range(4):
        Ub = gauge[mu][:, :, :, tsl]
        vb = np.stack([Ub.real, Ub.imag], axis=-3) * np.float32(-0.5)
        WB[mu] = np.ascontiguousarray(vb).reshape(XY, Z * TS * 18).astype(np.float16)
        if mu == 3:
            tf = [(t0 - 1 + i) % T for i in range(TS)]
            Uf = gauge[mu][:, :, :, tf]
        else:
            Uf = np.roll(gauge[mu], +1, axis=mu)[:, :, :, tsl]
        Vf = np.conjugate(np.swapaxes(Uf, -1, -2))
        vf = np.stack([Vf.real, Vf.imag], axis=-3) * np.float32(-0.5)
        WF[mu] = np.ascontiguousarray(vf).reshape(XY, Z * TS * 18).astype(np.float16)
    return {"psi_h": psi_h, "fi4": fi4, "WF": WF, "WB": WB}


def prep_in_maps(field, gauge):
    return [prep_core_inputs(field, gauge, k * TS) for k in range(NCORES)]


def assemble_output(res):
    out = np.empty((X, Y, Z, T, 3, 4), np.complex64)
    for k in range(NCORES):
        o = res[k]["outp"].reshape(X, Y, Z, TS, 4, 2, 3).astype(np.float32)
        oc = (o[..., 0, :] + 1j * o[..., 1, :]).transpose(0, 1, 2, 3, 5, 4)
        out[:, :, :, k * TS:(k + 1) * TS] = oc
    return out


def kernel(field, gauge_field):
    from concourse.bass_utils import run_bass_kernel_spmd

    if "v2" not in _CACHE:
        _CACHE["v2"] = build_module()
    nc = _CACHE["v2"]
    in_maps = prep_in_maps(np.asarray(field), np.asarray(gauge_field))
    res = run_bass_kernel_spmd(nc, in_maps, list(range(NCORES))).results
    return assemble_output(res)


# revision 8
# speedup vs baseline: 2.1177x; 1.2765x over previous
"""Wilson-Dirac operator on Trainium2, 8 NeuronCores, T-axis domain decomposition.

v3: all-f16 compute in PLANAR (SoA) layout — every SBUF tile is
[component-plane][z*t] with the site dimension contiguous innermost, so all
hot DVE ops stream long 144-element runs (measured: short strided runs cost
~7-8 cycles each; planar removes them). DVE 2x packed f16 mode throughout.

Host arrays per core (f16, gauge pre-scaled by -0.5, fwd links pre-rolled,
all component-plane-major per lattice row):
  psi_h [XY, 24, Z+2, TS+2]  psi slab, z+t halos, planes (s,p,c)
  fi4   [4, XY, 24, Z*TS]    interior psi pre-rolled (x+1, x-1, y+1, y-1)
  WF/WB [4, XY, 18, Z*TS]    hopping matrices, planes (p, A, B)
  outp  [XY, 24, Z*TS]       output, planes (s,p,c)

Blocks: 4 x (128 rows, z 0..24) + 1 x (64 rows, z halved across partitions).
Per (mu,sgn): proj h[j,p,b] -> products P[j,g,A,B] (h broadcast on outer A)
-> bsum over B (add1 on gpsimd, software-pipelined one term) -> combine
m[j,p,a] -> expand. Mass on ACT. Loads on SP queue, stores on ACT queue.
"""

import numpy as np

# ---------------------------------------------------------------- constants
X = Y = Z = 24
T = 48
NCORES = 8
TS = T // NCORES
TH = TS + 2
XY = X * Y
MASSP4 = 4.5

DIRSPEC = {
    0: dict(B=(3, 2), c=(-1j, -1j), e=(1, 0), d=(+1j, +1j)),
    1: dict(B=(3, 2), c=(-1, +1),   e=(1, 0), d=(+1, -1)),
    2: dict(B=(2, 3), c=(-1j, +1j), e=(0, 1), d=(+1j, -1j)),
    3: dict(B=(2, 3), c=(+1, +1),   e=(0, 1), d=(+1, +1)),
}

_CACHE = {}


def _split_waits_json(raw: bytes) -> bytes:
    """Walrus allows only ONE sync-wait per instruction: hoist extras onto
    NoOps inserted immediately before (same engine; sems monotonic => exact)."""
    import json
    bj = json.loads(raw)
    nid = 0
    for fn in bj.get("functions", []):
        for bb in fn.get("blocks", []):
            out = []
            changed = False
            for inst in bb.get("instructions", []):
                si = inst.get("sync_info")
                ow = (si or {}).get("on_wait") or []
                if len(ow) > 1:
                    changed = True
                    for w in ow[:-1]:
                        nid += 1
                        out.append({
                            "engine": inst["engine"], "ins": [], "outs": [],
                            "name": f"WSPL-{nid}", "opcode": "NoOp",
                            "sync_info": {"on_update": [], "on_wait": [w]},
                        })
                    si["on_wait"] = [ow[-1]]
                out.append(inst)
            if changed:
                bb["instructions"] = out
    return json.dumps(bj).encode()


def _install_json_wait_fix():
    import concourse.bass as bass
    if getattr(bass.Bass, "_wd_wait_fix", False):
        return
    orig = bass.Bass.to_json_bytes

    def patched(self, *a, **k):
        return _split_waits_json(orig(self, *a, **k))

    bass.Bass.to_json_bytes = patched
    bass.Bass._wd_wait_fix = True


def build_module(pool_add1=True):
    import concourse.bass as bass
    import concourse.mybir as mybir
    from concourse.ap import AP
    from concourse.mybir import AluOpType
    from concourse.tile import TileContext

    _install_json_wait_fix()
    F16 = mybir.dt.float16

    nc = bass.Bass()
    psi_h = nc.declare_dram_parameter("psi_h", [XY, 24, (Z + 2) * TH], F16, isOutput=False)
    fi4 = nc.declare_dram_parameter("fi4", [4, XY, 24, Z * TS], F16, isOutput=False)
    WFp = nc.declare_dram_parameter("WF", [4, XY, 18, Z * TS], F16, isOutput=False)
    WBp = nc.declare_dram_parameter("WB", [4, XY, 18, Z * TS], F16, isOutput=False)
    outp = nc.declare_dram_parameter("outp", [XY, 24, Z * TS], F16, isOutput=True)

    blocks = [(0, 128, [(0, 24, 0)]), (128, 128, [(0, 24, 0)]),
              (256, 128, [(0, 24, 0)]), (384, 128, [(0, 24, 0)]),
              (512, 64, [(0, 12, 0), (12, 12, 64)])]

    def sap(t, off, dims):
        return AP(t.tensor, t.offset + off, [list(t.ap[0])] + [list(d) for d in dims])

    with TileContext(nc) as tc:
        ctx_pool = tc.tile_pool(name="work", bufs=1)
        pool = ctx_pool.__enter__()
        V = nc.vector
        G = nc.gpsimd
        D = nc.sync
        ZFULL = Z * TS
        for (r0, R, parts) in blocks:
            zh = parts[0][1]
            npart = R * len(parts)
            ZT = zh * TS          # sites per row-slice
            PS = (zh + 2) * TH    # psi_al plane stride

            psi_al = pool.tile([npart, 24 * PS], F16, tag="psi_al", bufs=2)
            out_t = pool.tile([npart, 24 * ZT], F16, tag="out_t", bufs=2)
            # psi_h planar per-plane z-slice load (full-z: whole row)
            for (z0, _, p0) in parts:
                D.dma_start(out=psi_al[p0:p0 + R],
                            in_=psi_h[r0:r0 + R, :, z0 * TH:(z0 + zh + 2) * TH])

            def load(tag, src, mu, bufs, ncp):
                tl = pool.tile([npart, ncp * ZT], F16, tag=tag, bufs=bufs)
                for (z0, _, p0) in parts:
                    D.dma_start(out=tl[p0:p0 + R],
                                in_=src[mu, r0:r0 + R, :, z0 * TS:(z0 + zh) * TS])
                return tl

            # mass on ACT: out = 4.5 * psi interior (planes, z, t)
            nc.scalar.mul(
                sap(out_t, 0, [[ZT, 24], [6, zh], [1, 6]]),
                sap(psi_al, TH + 1, [[PS, 24], [TH, zh], [1, 6]]),
                MASSP4)

            tail_q = []

            def run_tail():
                if not tail_q:
                    return
                pt, st, mt, spec, dj = tail_q.pop()
                # bsum part 2 (add1 ran on gpsimd one term ago)
                V.tensor_tensor(sap(st, 0, [[ZT, 24], [1, ZT]]),
                                sap(st, 0, [[ZT, 24], [1, ZT]]),
                                sap(pt, 2 * ZT, [[3 * ZT, 24], [1, ZT]]),
                                AluOpType.add)
                # combine m[j,p,a]: m_re = S[g0]-S[g1]; m_im = S[g2]+S[g3]
                V.tensor_tensor(sap(mt, 0, [[6 * ZT, 2], [ZT, 3], [1, ZT]]),
                                sap(st, 0, [[12 * ZT, 2], [ZT, 3], [1, ZT]]),
                                sap(st, 3 * ZT, [[12 * ZT, 2], [ZT, 3], [1, ZT]]),
                                AluOpType.subtract)
                V.tensor_tensor(sap(mt, 3 * ZT, [[6 * ZT, 2], [ZT, 3], [1, ZT]]),
                                sap(st, 6 * ZT, [[12 * ZT, 2], [ZT, 3], [1, ZT]]),
                                sap(st, 9 * ZT, [[12 * ZT, 2], [ZT, 3], [1, ZT]]),
                                AluOpType.add)
                # expand
                d0, d1 = dj
                if d0.imag == 0.0 and d0.real == d1.real and spec["e"][0] == 0 \
                        and d0.real > 0:
                    # mu3 fwd: out[s] += m[s mod 2] for all 4 spins, one inst
                    V.tensor_tensor(sap(out_t, 0, [[12 * ZT, 2], [ZT, 12], [1, ZT]]),
                                    sap(out_t, 0, [[12 * ZT, 2], [ZT, 12], [1, ZT]]),
                                    sap(mt, 0, [[0, 2], [ZT, 12], [1, ZT]]),
                                    AluOpType.add)
                    return
                o01 = sap(out_t, 0, [[ZT, 12], [1, ZT]])
                V.tensor_tensor(o01, o01, sap(mt, 0, [[ZT, 12], [1, ZT]]),
                                AluOpType.add)
                if d0.imag == 0.0:
                    if d0.real == d1.real and spec["e"][0] == 0:
                        o23 = sap(out_t, 12 * ZT, [[ZT, 12], [1, ZT]])
                        V.tensor_tensor(o23, o23, sap(mt, 0, [[ZT, 12], [1, ZT]]),
                                        AluOpType.add if d0.real > 0 else AluOpType.subtract)
                    else:
                        for si, (e, dv) in enumerate(zip(spec["e"], dj)):
                            os_ = sap(out_t, (12 + 6 * si) * ZT, [[ZT, 6], [1, ZT]])
                            V.tensor_tensor(os_, os_, sap(mt, e * 6 * ZT, [[ZT, 6], [1, ZT]]),
                                            AluOpType.add if dv.real > 0 else AluOpType.subtract)
                else:
                    for si, (e, dv) in enumerate(zip(spec["e"], dj)):
                        sg = dv.imag > 0
                        ore = sap(out_t, (12 + 6 * si) * ZT, [[ZT, 3], [1, ZT]])
                        V.tensor_tensor(ore, ore,
                                        sap(mt, (e * 6 + 3) * ZT, [[ZT, 3], [1, ZT]]),
                                        AluOpType.subtract if sg else AluOpType.add)
                        oim = sap(out_t, (12 + 6 * si + 3) * ZT, [[ZT, 3], [1, ZT]])
                        V.tensor_tensor(oim, oim,
                                        sap(mt, e * 6 * ZT, [[ZT, 3], [1, ZT]]),
                                        AluOpType.add if sg else AluOpType.subtract)

            for mu in range(4):
                wf_t = load("w", WFp, mu, 6, 18)
                wb_t = load("w", WBp, mu, 6, 18)
                if mu <= 1:
                    pf_t = load("fi", fi4, 2 * mu, 8, 24)
                    pb_t = load("fi", fi4, 2 * mu + 1, 8, 24)
                spec = DIRSPEC[mu]

                for sgn in (+1, -1):
                    fwd = sgn > 0
                    cj = spec["c"] if fwd else tuple(-v for v in spec["c"])
                    dj = spec["d"] if fwd else tuple(-v for v in spec["d"])
                    wt = wf_t if fwd else wb_t

                    # psi source: planar planes; psi_al has (z,t) halo dims
                    if mu <= 1:
                        ps = pf_t if fwd else pb_t
                        pbase, pstr, pz = 0, ZT, [[1, ZT]]
                        hz = [[1, ZT]]
                    else:
                        if mu == 2:
                            pbase = (0 if fwd else 2 * TH) + 1
                        else:
                            pbase = TH + (0 if fwd else 2)
                        ps, pstr, pz = psi_al, PS, [[TH, zh], [1, 6]]
                        hz = [[6, zh], [1, 6]]

                    # --- proj h[j,p,b] = psi[A] + c*psi[B]  (plane-major)
                    ht = pool.tile([npart, 12 * ZT], F16, tag="h", bufs=2)
                    for j in (0, 1):
                        A, B, c = j, spec["B"][j], cj[j]
                        if c.imag == 0.0:
                            op = AluOpType.add if c.real > 0 else AluOpType.subtract
                            V.tensor_tensor(
                                sap(ht, j * 6 * ZT, [[ZT, 6]] + hz),
                                sap(ps, pbase + A * 6 * pstr, [[pstr, 6]] + pz),
                                sap(ps, pbase + B * 6 * pstr, [[pstr, 6]] + pz), op)
                        else:
                            sg = c.imag > 0
                            V.tensor_tensor(
                                sap(ht, j * 6 * ZT, [[ZT, 3]] + hz),
                                sap(ps, pbase + A * 6 * pstr, [[pstr, 3]] + pz),
                                sap(ps, pbase + (B * 6 + 3) * pstr, [[pstr, 3]] + pz),
                                AluOpType.subtract if sg else AluOpType.add)
                            V.tensor_tensor(
                                sap(ht, (j * 6 + 3) * ZT, [[ZT, 3]] + hz),
                                sap(ps, pbase + (A * 6 + 3) * pstr, [[pstr, 3]] + pz),
                                sap(ps, pbase + B * 6 * pstr, [[pstr, 3]] + pz),
                                AluOpType.add if sg else AluOpType.subtract)

                    # --- products P[j,g,A,B] planes = W[p,A,B] * h[j,p,B]
                    pt = pool.tile([npart, 72 * ZT], F16, tag="P", bufs=2)
                    for j in (0, 1):
                        for g, (wp, hp) in enumerate(((0, 0), (1, 1), (1, 0), (0, 1))):
                            V.tensor_tensor(
                                sap(pt, (j * 36 + g * 9) * ZT, [[3 * ZT, 3], [ZT, 3], [1, ZT]]),
                                sap(wt, wp * 9 * ZT, [[3 * ZT, 3], [ZT, 3], [1, ZT]]),
                                sap(ht, (j * 6 + hp * 3) * ZT, [[0, 3], [ZT, 3], [1, ZT]]),
                                AluOpType.mult)

                    # --- bsum part 1: S = P[B0] + P[B1]
                    st = pool.tile([npart, 24 * ZT], F16, tag="S", bufs=3)
                    E1 = G if pool_add1 else V
                    E1.tensor_tensor(sap(st, 0, [[ZT, 24], [1, ZT]]),
                                     sap(pt, 0, [[3 * ZT, 24], [1, ZT]]),
                                     sap(pt, ZT, [[3 * ZT, 24], [1, ZT]]),
                                     AluOpType.add)
                    mt = pool.tile([npart, 12 * ZT], F16, tag="m", bufs=3)

                    run_tail()
                    tail_q.append((pt, st, mt, spec, dj))

            run_tail()
            for (z0, _, p0) in parts:
                nc.scalar.dma_start(out=outp[r0:r0 + R, :, z0 * TS:(z0 + zh) * TS],
                                    in_=out_t[p0:p0 + R])
        ctx_pool.__exit__(None, None, None)
    return nc


# ---------------------------------------------------------------- host side
def prep_core_inputs(field, gauge, t0):
    """field [X,Y,Z,T,3,4] c64, gauge [4,X,Y,Z,T,3,3] c64 -> planar f16."""
    tsl = [(t0 + i) % T for i in range(TS)]
    th_idx = [(t0 - 1) % T] + tsl + [(t0 + TS) % T]
    f = field[:, :, :, th_idx]
    fr = np.stack([f.real, f.imag], axis=-1)            # [X,Y,Z,TH,c,s,p]
    fpl = fr.transpose(0, 1, 5, 6, 4, 2, 3)             # [X,Y,s,p,c,Z,TH]
    zhal = np.concatenate([fpl[..., -1:, :], fpl, fpl[..., :1, :]], axis=5)
    psi_h = np.ascontiguousarray(zhal).reshape(XY, 24 * (Z + 2) * TH).astype(np.float16)

    fin = fpl[..., :, 1:TS + 1]                         # [X,Y,s,p,c,Z,TS]
    rolls = [np.roll(fin, +1, 0), np.roll(fin, -1, 0),
             np.roll(fin, +1, 1), np.roll(fin, -1, 1)]
    fi4 = np.stack([np.ascontiguousarray(r).reshape(XY, 24 * Z * TS) for r in rolls]
                   ).astype(np.float16)

    WF = np.empty((4, XY, 18 * Z * TS), np.float16)
    WB = np.empty((4, XY, 18 * Z * TS), np.float16)
    for mu in range(4):
        Ub = gauge[mu][:, :, :, tsl]                    # [X,Y,Z,TS,A,B]
        vb = np.stack([Ub.real, Ub.imag], axis=4) * np.float32(-0.5)  # [X,Y,Z,TS,p,A,B]
        vbp = vb.transpose(0, 1, 4, 5, 6, 2, 3)         # [X,Y,p,A,B,Z,TS]
        WB[mu] = np.ascontiguousarray(vbp).reshape(XY, 18 * Z * TS).astype(np.float16)
        if mu == 3:
            tf = [(t0 - 1 + i) % T for i in range(TS)]
            Uf = gauge[mu][:, :, :, tf]
        else:
            Uf = np.roll(gauge[mu], +1, axis=mu)[:, :, :, tsl]
        Vf = np.conjugate(np.swapaxes(Uf, -1, -2))
        vf = np.stack([Vf.real, Vf.imag], axis=4) * np.float32(-0.5)
        vfp = vf.transpose(0, 1, 4, 5, 6, 2, 3)
        WF[mu] = np.ascontiguousarray(vfp).reshape(XY, 18 * Z * TS).astype(np.float16)
    return {"psi_h": psi_h, "fi4": fi4, "WF": WF, "WB": WB}


def prep_in_maps(field, gauge):
    return [prep_core_inputs(field, gauge, k * TS) for k in range(NCORES)]


def assemble_output(res):
    out = np.empty((X, Y, Z, T, 3, 4), np.complex64)
    for k in range(NCORES):
        o = res[k]["outp"].reshape(X, Y, 4, 2, 3, Z, TS).astype(np.float32)
        oc = (o[:, :, :, 0] + 1j * o[:, :, :, 1])       # [X,Y,s,c,Z,TS]
        out[:, :, :, k * TS:(k + 1) * TS] = oc.transpose(0, 1, 4, 5, 3, 2)
    return out


def kernel(field, gauge_field):
    from concourse.bass_utils import run_bass_kernel_spmd

    if "v3" not in _CACHE:
        _CACHE["v3"] = build_module()
    nc = _CACHE["v3"]
    in_maps = prep_in_maps(np.asarray(field), np.asarray(gauge_field))
    res = run_bass_kernel_spmd(nc, in_maps, list(range(NCORES))).results
    return assemble_output(res)


# revision 9
# speedup vs baseline: 2.1371x; 1.0092x over previous
"""Wilson-Dirac operator on Trainium2, 8 NeuronCores, T-axis domain decomposition.

v3: all-f16 compute in PLANAR (SoA) layout — every SBUF tile is
[component-plane][z*t] with the site dimension contiguous innermost, so all
hot DVE ops stream long 144-element runs (measured: short strided runs cost
~7-8 cycles each; planar removes them). DVE 2x packed f16 mode throughout.

Host arrays per core (f16, gauge pre-scaled by -0.5, fwd links pre-rolled,
all component-plane-major per lattice row):
  psi_h [XY, 24, Z+2, TS+2]  psi slab, z+t halos, planes (s,p,c)
  fi4   [4, XY, 24, Z*TS]    interior psi pre-rolled (x+1, x-1, y+1, y-1)
  WF/WB [4, XY, 18, Z*TS]    hopping matrices, planes (p, A, B)
  outp  [XY, 24, Z*TS]       output, planes (s,p,c)

Blocks: 4 x (128 rows, z 0..24) + 1 x (64 rows, z halved across partitions).
Per (mu,sgn): proj h[j,p,b] -> products P[j,g,A,B] (h broadcast on outer A)
-> bsum over B (add1 on gpsimd, software-pipelined one term) -> combine
m[j,p,a] -> expand. Mass on ACT. Loads on SP queue, stores on ACT queue.
"""

import numpy as np

# ---------------------------------------------------------------- constants
X = Y = Z = 24
T = 48
NCORES = 8
TS = T // NCORES
TH = TS + 2
XY = X * Y
MASSP4 = 4.5

DIRSPEC = {
    0: dict(B=(3, 2), c=(-1j, -1j), e=(1, 0), d=(+1j, +1j)),
    1: dict(B=(3, 2), c=(-1, +1),   e=(1, 0), d=(+1, -1)),
    2: dict(B=(2, 3), c=(-1j, +1j), e=(0, 1), d=(+1j, -1j)),
    3: dict(B=(2, 3), c=(+1, +1),   e=(0, 1), d=(+1, +1)),
}

_CACHE = {}


def _split_waits_json(raw: bytes) -> bytes:
    """Walrus allows only ONE sync-wait per instruction: hoist extras onto
    NoOps inserted immediately before (same engine; sems monotonic => exact)."""
    import json
    bj = json.loads(raw)
    nid = 0
    for fn in bj.get("functions", []):
        for bb in fn.get("blocks", []):
            out = []
            changed = False
            for inst in bb.get("instructions", []):
                si = inst.get("sync_info")
                ow = (si or {}).get("on_wait") or []
                if len(ow) > 1:
                    changed = True
                    for w in ow[:-1]:
                        nid += 1
                        out.append({
                            "engine": inst["engine"], "ins": [], "outs": [],
                            "name": f"WSPL-{nid}", "opcode": "NoOp",
                            "sync_info": {"on_update": [], "on_wait": [w]},
                        })
                    si["on_wait"] = [ow[-1]]
                out.append(inst)
            if changed:
                bb["instructions"] = out
    return json.dumps(bj).encode()


def _install_json_wait_fix():
    import concourse.bass as bass
    if getattr(bass.Bass, "_wd_wait_fix", False):
        return
    orig = bass.Bass.to_json_bytes

    def patched(self, *a, **k):
        return _split_waits_json(orig(self, *a, **k))

    bass.Bass.to_json_bytes = patched
    bass.Bass._wd_wait_fix = True


def build_module(pool_add1=False):
    import concourse.bass as bass
    import concourse.mybir as mybir
    from concourse.ap import AP
    from concourse.mybir import AluOpType
    from concourse.tile import TileContext

    _install_json_wait_fix()
    F16 = mybir.dt.float16

    nc = bass.Bass()
    psi_h = nc.declare_dram_parameter("psi_h", [XY, 24, (Z + 2) * TH], F16, isOutput=False)
    fi4 = nc.declare_dram_parameter("fi4", [4, XY, 24, Z * TS], F16, isOutput=False)
    WFp = nc.declare_dram_parameter("WF", [4, XY, 18, Z * TS], F16, isOutput=False)
    WBp = nc.declare_dram_parameter("WB", [4, XY, 18, Z * TS], F16, isOutput=False)
    outp = nc.declare_dram_parameter("outp", [XY, 24, Z * TS], F16, isOutput=True)

    blocks = [(0, 128, [(0, 24, 0)]), (128, 128, [(0, 24, 0)]),
              (256, 128, [(0, 24, 0)]), (384, 128, [(0, 24, 0)]),
              (512, 64, [(0, 12, 0), (12, 12, 64)])]

    def sap(t, off, dims):
        return AP(t.tensor, t.offset + off, [list(t.ap[0])] + [list(d) for d in dims])

    with TileContext(nc) as tc:
        ctx_pool = tc.tile_pool(name="work", bufs=1)
        pool = ctx_pool.__enter__()
        V = nc.vector
        G = nc.gpsimd
        D = nc.sync
        ZFULL = Z * TS
        for (r0, R, parts) in blocks:
            zh = parts[0][1]
            npart = R * len(parts)
            ZT = zh * TS          # sites per row-slice
            PS = (zh + 2) * TH    # psi_al plane stride

            psi_al = pool.tile([npart, 24 * PS], F16, tag="psi_al", bufs=2)
            out_t = pool.tile([npart, 24 * ZT], F16, tag="out_t", bufs=2)
            # psi_h planar per-plane z-slice load (full-z: whole row)
            for (z0, _, p0) in parts:
                D.dma_start(out=psi_al[p0:p0 + R],
                            in_=psi_h[r0:r0 + R, :, z0 * TH:(z0 + zh + 2) * TH])

            def load(tag, src, mu, bufs, ncp):
                tl = pool.tile([npart, ncp * ZT], F16, tag=tag, bufs=bufs)
                for (z0, _, p0) in parts:
                    D.dma_start(out=tl[p0:p0 + R],
                                in_=src[mu, r0:r0 + R, :, z0 * TS:(z0 + zh) * TS])
                return tl

            # mass on ACT: out = 4.5 * psi interior (planes, z, t)
            nc.scalar.mul(
                sap(out_t, 0, [[ZT, 24], [6, zh], [1, 6]]),
                sap(psi_al, TH + 1, [[PS, 24], [TH, zh], [1, 6]]),
                MASSP4)

            tail_q = []

            EX = G
            def run_tail():
                if not tail_q:
                    return
                pt, st, mt, spec, dj = tail_q.pop()
                # bsum part 2 (add1 ran on gpsimd one term ago)
                V.tensor_tensor(sap(st, 0, [[ZT, 24], [1, ZT]]),
                                sap(st, 0, [[ZT, 24], [1, ZT]]),
                                sap(pt, 2 * ZT, [[3 * ZT, 24], [1, ZT]]),
                                AluOpType.add)
                # combine m[j,p,a]: m_re = S[g0]-S[g1]; m_im = S[g2]+S[g3]
                V.tensor_tensor(sap(mt, 0, [[6 * ZT, 2], [ZT, 3], [1, ZT]]),
                                sap(st, 0, [[12 * ZT, 2], [ZT, 3], [1, ZT]]),
                                sap(st, 3 * ZT, [[12 * ZT, 2], [ZT, 3], [1, ZT]]),
                                AluOpType.subtract)
                V.tensor_tensor(sap(mt, 3 * ZT, [[6 * ZT, 2], [ZT, 3], [1, ZT]]),
                                sap(st, 6 * ZT, [[12 * ZT, 2], [ZT, 3], [1, ZT]]),
                                sap(st, 9 * ZT, [[12 * ZT, 2], [ZT, 3], [1, ZT]]),
                                AluOpType.add)
                # expand
                d0, d1 = dj
                if d0.imag == 0.0 and d0.real == d1.real and spec["e"][0] == 0 \
                        and d0.real > 0:
                    # mu3 fwd: out[s] += m[s mod 2] for all 4 spins, one inst
                    EX.tensor_tensor(sap(out_t, 0, [[12 * ZT, 2], [ZT, 12], [1, ZT]]),
                                    sap(out_t, 0, [[12 * ZT, 2], [ZT, 12], [1, ZT]]),
                                    sap(mt, 0, [[0, 2], [ZT, 12], [1, ZT]]),
                                    AluOpType.add)
                    return
                o01 = sap(out_t, 0, [[ZT, 12], [1, ZT]])
                EX.tensor_tensor(o01, o01, sap(mt, 0, [[ZT, 12], [1, ZT]]),
                                AluOpType.add)
                if d0.imag == 0.0:
                    if d0.real == d1.real and spec["e"][0] == 0:
                        o23 = sap(out_t, 12 * ZT, [[ZT, 12], [1, ZT]])
                        EX.tensor_tensor(o23, o23, sap(mt, 0, [[ZT, 12], [1, ZT]]),
                                        AluOpType.add if d0.real > 0 else AluOpType.subtract)
                    else:
                        for si, (e, dv) in enumerate(zip(spec["e"], dj)):
                            os_ = sap(out_t, (12 + 6 * si) * ZT, [[ZT, 6], [1, ZT]])
                            EX.tensor_tensor(os_, os_, sap(mt, e * 6 * ZT, [[ZT, 6], [1, ZT]]),
                                            AluOpType.add if dv.real > 0 else AluOpType.subtract)
                else:
                    for si, (e, dv) in enumerate(zip(spec["e"], dj)):
                        sg = dv.imag > 0
                        ore = sap(out_t, (12 + 6 * si) * ZT, [[ZT, 3], [1, ZT]])
                        EX.tensor_tensor(ore, ore,
                                        sap(mt, (e * 6 + 3) * ZT, [[ZT, 3], [1, ZT]]),
                                        AluOpType.subtract if sg else AluOpType.add)
                        oim = sap(out_t, (12 + 6 * si + 3) * ZT, [[ZT, 3], [1, ZT]])
                        EX.tensor_tensor(oim, oim,
                                        sap(mt, e * 6 * ZT, [[ZT, 3], [1, ZT]]),
                                        AluOpType.add if sg else AluOpType.subtract)

            for mu in range(4):
                wf_t = load("w", WFp, mu, 6, 18)
                wb_t = load("w", WBp, mu, 6, 18)
                if mu <= 1:
                    pf_t = load("fi", fi4, 2 * mu, 8, 24)
                    pb_t = load("fi", fi4, 2 * mu + 1, 8, 24)
                spec = DIRSPEC[mu]

                for sgn in (+1, -1):
                    fwd = sgn > 0
                    cj = spec["c"] if fwd else tuple(-v for v in spec["c"])
                    dj = spec["d"] if fwd else tuple(-v for v in spec["d"])
                    wt = wf_t if fwd else wb_t

                    # psi source: planar planes; psi_al has (z,t) halo dims
                    if mu <= 1:
                        ps = pf_t if fwd else pb_t
                        pbase, pstr, pz = 0, ZT, [[1, ZT]]
                        hz = [[1, ZT]]
                    else:
                        if mu == 2:
                            pbase = (0 if fwd else 2 * TH) + 1
                        else:
                            pbase = TH + (0 if fwd else 2)
                        ps, pstr, pz = psi_al, PS, [[TH, zh], [1, 6]]
                        hz = [[6, zh], [1, 6]]

                    # --- proj h[j,p,b] = psi[A] + c*psi[B]  (plane-major)
                    ht = pool.tile([npart, 12 * ZT], F16, tag="h", bufs=2)
                    for j in (0, 1):
                        A, B, c = j, spec["B"][j], cj[j]
                        if c.imag == 0.0:
                            op = AluOpType.add if c.real > 0 else AluOpType.subtract
                            V.tensor_tensor(
                                sap(ht, j * 6 * ZT, [[ZT, 6]] + hz),
                                sap(ps, pbase + A * 6 * pstr, [[pstr, 6]] + pz),
                                sap(ps, pbase + B * 6 * pstr, [[pstr, 6]] + pz), op)
                        else:
                            sg = c.imag > 0
                            V.tensor_tensor(
                                sap(ht, j * 6 * ZT, [[ZT, 3]] + hz),
                                sap(ps, pbase + A * 6 * pstr, [[pstr, 3]] + pz),
                                sap(ps, pbase + (B * 6 + 3) * pstr, [[pstr, 3]] + pz),
                                AluOpType.subtract if sg else AluOpType.add)
                            V.tensor_tensor(
                                sap(ht, (j * 6 + 3) * ZT, [[ZT, 3]] + hz),
                                sap(ps, pbase + (A * 6 + 3) * pstr, [[pstr, 3]] + pz),
                                sap(ps, pbase + B * 6 * pstr, [[pstr, 3]] + pz),
                                AluOpType.add if sg else AluOpType.subtract)

                    # --- products P[j,g,A,B] planes = W[p,A,B] * h[j,p,B]
                    pt = pool.tile([npart, 72 * ZT], F16, tag="P", bufs=2)
                    for j in (0, 1):
                        for g, (wp, hp) in enumerate(((0, 0), (1, 1), (1, 0), (0, 1))):
                            V.tensor_tensor(
                                sap(pt, (j * 36 + g * 9) * ZT, [[3 * ZT, 3], [ZT, 3], [1, ZT]]),
                                sap(wt, wp * 9 * ZT, [[3 * ZT, 3], [ZT, 3], [1, ZT]]),
                                sap(ht, (j * 6 + hp * 3) * ZT, [[0, 3], [ZT, 3], [1, ZT]]),
                                AluOpType.mult)

                    # --- bsum part 1: S = P[B0] + P[B1]
                    st = pool.tile([npart, 24 * ZT], F16, tag="S", bufs=3)
                    E1 = G if pool_add1 else V
                    E1.tensor_tensor(sap(st, 0, [[ZT, 24], [1, ZT]]),
                                     sap(pt, 0, [[3 * ZT, 24], [1, ZT]]),
                                     sap(pt, ZT, [[3 * ZT, 24], [1, ZT]]),
                                     AluOpType.add)
                    mt = pool.tile([npart, 12 * ZT], F16, tag="m", bufs=3)

                    run_tail()
                    tail_q.append((pt, st, mt, spec, dj))

            run_tail()
            for (z0, _, p0) in parts:
                nc.scalar.dma_start(out=outp[r0:r0 + R, :, z0 * TS:(z0 + zh) * TS],
                                    in_=out_t[p0:p0 + R])
        ctx_pool.__exit__(None, None, None)
    return nc


# ---------------------------------------------------------------- host side
def prep_core_inputs(field, gauge, t0):
    """field [X,Y,Z,T,3,4] c64, gauge [4,X,Y,Z,T,3,3] c64 -> planar f16."""
    tsl = [(t0 + i) % T for i in range(TS)]
    th_idx = [(t0 - 1) % T] + tsl + [(t0 + TS) % T]
    f = field[:, :, :, th_idx]
    fr = np.stack([f.real, f.imag], axis=-1)            # [X,Y,Z,TH,c,s,p]
    fpl = fr.transpose(0, 1, 5, 6, 4, 2, 3)             # [X,Y,s,p,c,Z,TH]
    zhal = np.concatenate([fpl[..., -1:, :], fpl, fpl[..., :1, :]], axis=5)
    psi_h = np.ascontiguousarray(zhal).reshape(XY, 24 * (Z + 2) * TH).astype(np.float16)

    fin = fpl[..., :, 1:TS + 1]                         # [X,Y,s,p,c,Z,TS]
    rolls = [np.roll(fin, +1, 0), np.roll(fin, -1, 0),
             np.roll(fin, +1, 1), np.roll(fin, -1, 1)]
    fi4 = np.stack([np.ascontiguousarray(r).reshape(XY, 24 * Z * TS) for r in rolls]
                   ).astype(np.float16)

    WF = np.empty((4, XY, 18 * Z * TS), np.float16)
    WB = np.empty((4, XY, 18 * Z * TS), np.float16)
    for mu in range(4):
        Ub = gauge[mu][:, :, :, tsl]                    # [X,Y,Z,TS,A,B]
        vb = np.stack([Ub.real, Ub.imag], axis=4) * np.float32(-0.5)  # [X,Y,Z,TS,p,A,B]
        vbp = vb.transpose(0, 1, 4, 5, 6, 2, 3)         # [X,Y,p,A,B,Z,TS]
        WB[mu] = np.ascontiguousarray(vbp).reshape(XY, 18 * Z * TS).astype(np.float16)
        if mu == 3:
            tf = [(t0 - 1 + i) % T for i in range(TS)]
            Uf = gauge[mu][:, :, :, tf]
        else:
            Uf = np.roll(gauge[mu], +1, axis=mu)[:, :, :, tsl]
        Vf = np.conjugate(np.swapaxes(Uf, -1, -2))
        vf = np.stack([Vf.real, Vf.imag], axis=4) * np.float32(-0.5)
        vfp = vf.transpose(0, 1, 4, 5, 6, 2, 3)
        WF[mu] = np.ascontiguousarray(vfp).reshape(XY, 18 * Z * TS).astype(np.float16)
    return {"psi_h": psi_h, "fi4": fi4, "WF": WF, "WB": WB}


def prep_in_maps(field, gauge):
    return [prep_core_inputs(field, gauge, k * TS) for k in range(NCORES)]


def assemble_output(res):
    out = np.empty((X, Y, Z, T, 3, 4), np.complex64)
    for k in range(NCORES):
        o = res[k]["outp"].reshape(X, Y, 4, 2, 3, Z, TS).astype(np.float32)
        oc = (o[:, :, :, 0] + 1j * o[:, :, :, 1])       # [X,Y,s,c,Z,TS]
        out[:, :, :, k * TS:(k + 1) * TS] = oc.transpose(0, 1, 4, 5, 3, 2)
    return out


def kernel(field, gauge_field):
    from concourse.bass_utils import run_bass_kernel_spmd

    if "v3" not in _CACHE:
        _CACHE["v3"] = build_module()
    nc = _CACHE["v3"]
    in_maps = prep_in_maps(np.asarray(field), np.asarray(gauge_field))
    res = run_bass_kernel_spmd(nc, in_maps, list(range(NCORES))).results
    return assemble_output(res)


# revision 10
# speedup vs baseline: 2.1397x; 1.0012x over previous
"""Wilson-Dirac operator on Trainium2, 8 NeuronCores, T-axis domain decomposition.

v3: all-f16 compute in PLANAR (SoA) layout — every SBUF tile is
[component-plane][z*t] with the site dimension contiguous innermost, so all
hot DVE ops stream long 144-element runs (measured: short strided runs cost
~7-8 cycles each; planar removes them). DVE 2x packed f16 mode throughout.

Host arrays per core (f16, gauge pre-scaled by -0.5, fwd links pre-rolled,
all component-plane-major per lattice row):
  psi_h [XY, 24, Z+2, TS+2]  psi slab, z+t halos, planes (s,p,c)
  fi4   [4, XY, 24, Z*TS]    interior psi pre-rolled (x+1, x-1, y+1, y-1)
  WF/WB [4, XY, 18, Z*TS]    hopping matrices, planes (p, A, B)
  outp  [XY, 24, Z*TS]       output, planes (s,p,c)

Blocks: 4 x (128 rows, z 0..24) + 1 x (64 rows, z halved across partitions).
Per (mu,sgn): proj h[j,p,b] -> products P[j,g,A,B] (h broadcast on outer A)
-> bsum over B (add1 on gpsimd, software-pipelined one term) -> combine
m[j,p,a] -> expand. Mass on ACT. Loads on SP queue, stores on ACT queue.
"""

import numpy as np

# ---------------------------------------------------------------- constants
X = Y = Z = 24
T = 48
NCORES = 8
TS = T // NCORES
TH = TS + 2
XY = X * Y
MASSP4 = 4.5

DIRSPEC = {
    0: dict(B=(3, 2), c=(-1j, -1j), e=(1, 0), d=(+1j, +1j)),
    1: dict(B=(3, 2), c=(-1, +1),   e=(1, 0), d=(+1, -1)),
    2: dict(B=(2, 3), c=(-1j, +1j), e=(0, 1), d=(+1j, -1j)),
    3: dict(B=(2, 3), c=(+1, +1),   e=(0, 1), d=(+1, +1)),
}

_CACHE = {}


def _split_waits_json(raw: bytes) -> bytes:
    """Walrus allows only ONE sync-wait per instruction: hoist extras onto
    NoOps inserted immediately before (same engine; sems monotonic => exact)."""
    import json
    bj = json.loads(raw)
    nid = 0
    for fn in bj.get("functions", []):
        for bb in fn.get("blocks", []):
            out = []
            changed = False
            for inst in bb.get("instructions", []):
                si = inst.get("sync_info")
                ow = (si or {}).get("on_wait") or []
                if len(ow) > 1:
                    changed = True
                    for w in ow[:-1]:
                        nid += 1
                        out.append({
                            "engine": inst["engine"], "ins": [], "outs": [],
                            "name": f"WSPL-{nid}", "opcode": "NoOp",
                            "sync_info": {"on_update": [], "on_wait": [w]},
                        })
                    si["on_wait"] = [ow[-1]]
                out.append(inst)
            if changed:
                bb["instructions"] = out
    return json.dumps(bj).encode()


def _install_json_wait_fix():
    import concourse.bass as bass
    if getattr(bass.Bass, "_wd_wait_fix", False):
        return
    orig = bass.Bass.to_json_bytes

    def patched(self, *a, **k):
        return _split_waits_json(orig(self, *a, **k))

    bass.Bass.to_json_bytes = patched
    bass.Bass._wd_wait_fix = True


def build_module(pool_add1=False):
    import concourse.bass as bass
    import concourse.mybir as mybir
    from concourse.ap import AP
    from concourse.mybir import AluOpType
    from concourse.tile import TileContext

    _install_json_wait_fix()
    F16 = mybir.dt.float16

    nc = bass.Bass()
    psi_h = nc.declare_dram_parameter("psi_h", [XY, 24, (Z + 2) * TH], F16, isOutput=False)
    fi4 = nc.declare_dram_parameter("fi4", [4, XY, 24, Z * TS], F16, isOutput=False)
    WFp = nc.declare_dram_parameter("WF", [4, XY, 18, Z * TS], F16, isOutput=False)
    WBp = nc.declare_dram_parameter("WB", [4, XY, 18, Z * TS], F16, isOutput=False)
    outp = nc.declare_dram_parameter("outp", [XY, 24, Z * TS], F16, isOutput=True)

    blocks = [(0, 128, [(0, 24, 0)]), (128, 128, [(0, 24, 0)]),
              (256, 128, [(0, 24, 0)]), (384, 128, [(0, 24, 0)]),
              (512, 64, [(0, 12, 0), (12, 12, 64)])]

    def sap(t, off, dims):
        return AP(t.tensor, t.offset + off, [list(t.ap[0])] + [list(d) for d in dims])

    with TileContext(nc) as tc:
        ctx_pool = tc.tile_pool(name="work", bufs=1)
        pool = ctx_pool.__enter__()
        V = nc.vector
        G = nc.gpsimd
        D = nc.sync
        ZFULL = Z * TS
        for (r0, R, parts) in blocks:
            zh = parts[0][1]
            npart = R * len(parts)
            ZT = zh * TS          # sites per row-slice
            PS = (zh + 2) * TH    # psi_al plane stride

            psi_al = pool.tile([npart, 24 * PS], F16, tag="psi_al", bufs=2)
            out_t = pool.tile([npart, 24 * ZT], F16, tag="out_t", bufs=2)
            # psi_h planar per-plane z-slice load (full-z: whole row)
            for (z0, _, p0) in parts:
                D.dma_start(out=psi_al[p0:p0 + R],
                            in_=psi_h[r0:r0 + R, :, z0 * TH:(z0 + zh + 2) * TH])

            def load(tag, src, mu, bufs, ncp):
                tl = pool.tile([npart, ncp * ZT], F16, tag=tag, bufs=bufs)
                for (z0, _, p0) in parts:
                    D.dma_start(out=tl[p0:p0 + R],
                                in_=src[mu, r0:r0 + R, :, z0 * TS:(z0 + zh) * TS])
                return tl

            # mass on ACT: out = 4.5 * psi interior (planes, z, t)
            nc.scalar.mul(
                sap(out_t, 0, [[ZT, 24], [6, zh], [1, 6]]),
                sap(psi_al, TH + 1, [[PS, 24], [TH, zh], [1, 6]]),
                MASSP4)

            tail_q = []

            EX = G
            def run_tail():
                if not tail_q:
                    return
                pt, st, mt, spec, dj = tail_q.pop()
                # bsum part 2 (add1 ran on gpsimd one term ago)
                V.tensor_tensor(sap(st, 0, [[ZT, 24], [1, ZT]]),
                                sap(st, 0, [[ZT, 24], [1, ZT]]),
                                sap(pt, 2 * ZT, [[3 * ZT, 24], [1, ZT]]),
                                AluOpType.add)
                # combine m[j,p,a]: m_re = S[g0]-S[g1]; m_im = S[g2]+S[g3]
                V.tensor_tensor(sap(mt, 0, [[6 * ZT, 2], [ZT, 3], [1, ZT]]),
                                sap(st, 0, [[12 * ZT, 2], [ZT, 3], [1, ZT]]),
                                sap(st, 3 * ZT, [[12 * ZT, 2], [ZT, 3], [1, ZT]]),
                                AluOpType.subtract)
                V.tensor_tensor(sap(mt, 3 * ZT, [[6 * ZT, 2], [ZT, 3], [1, ZT]]),
                                sap(st, 6 * ZT, [[12 * ZT, 2], [ZT, 3], [1, ZT]]),
                                sap(st, 9 * ZT, [[12 * ZT, 2], [ZT, 3], [1, ZT]]),
                                AluOpType.add)
                # expand
                d0, d1 = dj
                if d0.imag == 0.0 and d0.real == d1.real and spec["e"][0] == 0 \
                        and d0.real > 0:
                    # mu3 fwd: out[s] += m[s mod 2] for all 4 spins, one inst
                    EX.tensor_tensor(sap(out_t, 0, [[12 * ZT, 2], [ZT, 12], [1, ZT]]),
                                    sap(out_t, 0, [[12 * ZT, 2], [ZT, 12], [1, ZT]]),
                                    sap(mt, 0, [[0, 2], [ZT, 12], [1, ZT]]),
                                    AluOpType.add)
                    return
                o01 = sap(out_t, 0, [[ZT, 12], [1, ZT]])
                EX.tensor_tensor(o01, o01, sap(mt, 0, [[ZT, 12], [1, ZT]]),
                                AluOpType.add)
                if d0.imag == 0.0:
                    if d0.real == d1.real and spec["e"][0] == 0:
                        o23 = sap(out_t, 12 * ZT, [[ZT, 12], [1, ZT]])
                        EX.tensor_tensor(o23, o23, sap(mt, 0, [[ZT, 12], [1, ZT]]),
                                        AluOpType.add if d0.real > 0 else AluOpType.subtract)
                    else:
                        for si, (e, dv) in enumerate(zip(spec["e"], dj)):
                            os_ = sap(out_t, (12 + 6 * si) * ZT, [[ZT, 6], [1, ZT]])
                            EX.tensor_tensor(os_, os_, sap(mt, e * 6 * ZT, [[ZT, 6], [1, ZT]]),
                                            AluOpType.add if dv.real > 0 else AluOpType.subtract)
                else:
                    for si, (e, dv) in enumerate(zip(spec["e"], dj)):
                        sg = dv.imag > 0
                        ore = sap(out_t, (12 + 6 * si) * ZT, [[ZT, 3], [1, ZT]])
                        EX.tensor_tensor(ore, ore,
                                        sap(mt, (e * 6 + 3) * ZT, [[ZT, 3], [1, ZT]]),
                                        AluOpType.subtract if sg else AluOpType.add)
                        oim = sap(out_t, (12 + 6 * si + 3) * ZT, [[ZT, 3], [1, ZT]])
                        EX.tensor_tensor(oim, oim,
                                        sap(mt, e * 6 * ZT, [[ZT, 3], [1, ZT]]),
                                        AluOpType.add if sg else AluOpType.subtract)

            for mu in range(4):
                wf_t = load("w", WFp, mu, 6, 18)
                wb_t = load("w", WBp, mu, 6, 18)
                if mu <= 1:
                    pf_t = load("fi", fi4, 2 * mu, 8, 24)
                    pb_t = load("fi", fi4, 2 * mu + 1, 8, 24)
                spec = DIRSPEC[mu]

                for sgn in (+1, -1):
                    fwd = sgn > 0
                    cj = spec["c"] if fwd else tuple(-v for v in spec["c"])
                    dj = spec["d"] if fwd else tuple(-v for v in spec["d"])
                    wt = wf_t if fwd else wb_t

                    # psi source: planar planes; psi_al has (z,t) halo dims
                    if mu <= 1:
                        ps = pf_t if fwd else pb_t
                        pbase, pstr, pz = 0, ZT, [[1, ZT]]
                        hz = [[1, ZT]]
                    else:
                        if mu == 2:
                            pbase = (0 if fwd else 2 * TH) + 1
                        else:
                            pbase = TH + (0 if fwd else 2)
                        ps, pstr, pz = psi_al, PS, [[TH, zh], [1, 6]]
                        hz = [[6, zh], [1, 6]]

                    # --- proj h[j,p,b] = psi[A] + c*psi[B]  (plane-major)
                    ht = pool.tile([npart, 12 * ZT], F16, tag="h", bufs=2)
                    for j in (0, 1):
                        A, B, c = j, spec["B"][j], cj[j]
                        if c.imag == 0.0:
                            op = AluOpType.add if c.real > 0 else AluOpType.subtract
                            V.tensor_tensor(
                                sap(ht, j * 6 * ZT, [[ZT, 6]] + hz),
                                sap(ps, pbase + A * 6 * pstr, [[pstr, 6]] + pz),
                                sap(ps, pbase + B * 6 * pstr, [[pstr, 6]] + pz), op)
                        else:
                            sg = c.imag > 0
                            V.tensor_tensor(
                                sap(ht, j * 6 * ZT, [[ZT, 3]] + hz),
                                sap(ps, pbase + A * 6 * pstr, [[pstr, 3]] + pz),
                                sap(ps, pbase + (B * 6 + 3) * pstr, [[pstr, 3]] + pz),
                                AluOpType.subtract if sg else AluOpType.add)
                            V.tensor_tensor(
                                sap(ht, (j * 6 + 3) * ZT, [[ZT, 3]] + hz),
                                sap(ps, pbase + (A * 6 + 3) * pstr, [[pstr, 3]] + pz),
                                sap(ps, pbase + B * 6 * pstr, [[pstr, 3]] + pz),
                                AluOpType.add if sg else AluOpType.subtract)

                    # --- products P[j,g,A,B] planes = W[p,A,B] * h[j,p,B]
                    pt = pool.tile([npart, 72 * ZT], F16, tag="P", bufs=2)
                    for j in (0, 1):
                        for g, (wp, hp) in enumerate(((0, 0), (1, 1), (1, 0), (0, 1))):
                            V.tensor_tensor(
                                sap(pt, (j * 36 + g * 9) * ZT, [[3 * ZT, 3], [ZT, 3], [1, ZT]]),
                                sap(wt, wp * 9 * ZT, [[3 * ZT, 3], [ZT, 3], [1, ZT]]),
                                sap(ht, (j * 6 + hp * 3) * ZT, [[0, 3], [ZT, 3], [1, ZT]]),
                                AluOpType.mult)

                    # --- bsum part 1: S = P[B0] + P[B1]
                    st = pool.tile([npart, 24 * ZT], F16, tag="S", bufs=3)
                    E1 = G if pool_add1 else V
                    E1.tensor_tensor(sap(st, 0, [[ZT, 24], [1, ZT]]),
                                     sap(pt, 0, [[3 * ZT, 24], [1, ZT]]),
                                     sap(pt, ZT, [[3 * ZT, 24], [1, ZT]]),
                                     AluOpType.add)
                    mt = pool.tile([npart, 12 * ZT], F16, tag="m", bufs=6)

                    run_tail()
                    tail_q.append((pt, st, mt, spec, dj))

            run_tail()
            for (z0, _, p0) in parts:
                nc.scalar.dma_start(out=outp[r0:r0 + R, :, z0 * TS:(z0 + zh) * TS],
                                    in_=out_t[p0:p0 + R])
        ctx_pool.__exit__(None, None, None)
    return nc


# ---------------------------------------------------------------- host side
def prep_core_inputs(field, gauge, t0):
    """field [X,Y,Z,T,3,4] c64, gauge [4,X,Y,Z,T,3,3] c64 -> planar f16."""
    tsl = [(t0 + i) % T for i in range(TS)]
    th_idx = [(t0 - 1) % T] + tsl + [(t0 + TS) % T]
    f = field[:, :, :, th_idx]
    fr = np.stack([f.real, f.imag], axis=-1)            # [X,Y,Z,TH,c,s,p]
    fpl = fr.transpose(0, 1, 5, 6, 4, 2, 3)             # [X,Y,s,p,c,Z,TH]
    zhal = np.concatenate([fpl[..., -1:, :], fpl, fpl[..., :1, :]], axis=5)
    psi_h = np.ascontiguousarray(zhal).reshape(XY, 24 * (Z + 2) * TH).astype(np.float16)

    fin = fpl[..., :, 1:TS + 1]                         # [X,Y,s,p,c,Z,TS]
    rolls = [np.roll(fin, +1, 0), np.roll(fin, -1, 0),
             np.roll(fin, +1, 1), np.roll(fin, -1, 1)]
    fi4 = np.stack([np.ascontiguousarray(r).reshape(XY, 24 * Z * TS) for r in rolls]
                   ).astype(np.float16)

    WF = np.empty((4, XY, 18 * Z * TS), np.float16)
    WB = np.empty((4, XY, 18 * Z * TS), np.float16)
    for mu in range(4):
        Ub = gauge[mu][:, :, :, tsl]                    # [X,Y,Z,TS,A,B]
        vb = np.stack([Ub.real, Ub.imag], axis=4) * np.float32(-0.5)  # [X,Y,Z,TS,p,A,B]
        vbp = vb.transpose(0, 1, 4, 5, 6, 2, 3)         # [X,Y,p,A,B,Z,TS]
        WB[mu] = np.ascontiguousarray(vbp).reshape(XY, 18 * Z * TS).astype(np.float16)
        if mu == 3:
            tf = [(t0 - 1 + i) % T for i in range(TS)]
            Uf = gauge[mu][:, :, :, tf]
        else:
            Uf = np.roll(gauge[mu], +1, axis=mu)[:, :, :, tsl]
        Vf = np.conjugate(np.swapaxes(Uf, -1, -2))
        vf = np.stack([Vf.real, Vf.imag], axis=4) * np.float32(-0.5)
        vfp = vf.transpose(0, 1, 4, 5, 6, 2, 3)
        WF[mu] = np.ascontiguousarray(vfp).reshape(XY, 18 * Z * TS).astype(np.float16)
    return {"psi_h": psi_h, "fi4": fi4, "WF": WF, "WB": WB}


def prep_in_maps(field, gauge):
    return [prep_core_inputs(field, gauge, k * TS) for k in range(NCORES)]


def assemble_output(res):
    out = np.empty((X, Y, Z, T, 3, 4), np.complex64)
    for k in range(NCORES):
        o = res[k]["outp"].reshape(X, Y, 4, 2, 3, Z, TS).astype(np.float32)
        oc = (o[:, :, :, 0] + 1j * o[:, :, :, 1])       # [X,Y,s,c,Z,TS]
        out[:, :, :, k * TS:(k + 1) * TS] = oc.transpose(0, 1, 4, 5, 3, 2)
    return out


def kernel(field, gauge_field):
    from concourse.bass_utils import run_bass_kernel_spmd

    if "v3" not in _CACHE:
        _CACHE["v3"] = build_module()
    nc = _CACHE["v3"]
    in_maps = prep_in_maps(np.asarray(field), np.asarray(gauge_field))
    res = run_bass_kernel_spmd(nc, in_maps, list(range(NCORES))).results
    return assemble_output(res)


# revision 11
# speedup vs baseline: 2.7906x; 1.3042x over previous
"""Wilson-Dirac operator on Trainium2, 8 NeuronCores, T-axis domain decomposition.

v3: all-f16 compute in PLANAR (SoA) layout — every SBUF tile is
[component-plane][z*t] with the site dimension contiguous innermost, so all
hot DVE ops stream long 144-element runs (measured: short strided runs cost
~7-8 cycles each; planar removes them). DVE 2x packed f16 mode throughout.

Host arrays per core (f16, gauge pre-scaled by -0.5, fwd links pre-rolled,
all component-plane-major per lattice row):
  psi_h [XY, 24, Z+2, TS+2]  psi slab, z+t halos, planes (s,p,c)
  fi4   [4, XY, 24, Z*TS]    interior psi pre-rolled (x+1, x-1, y+1, y-1)
  WF/WB [4, XY, 18, Z*TS]    hopping matrices, planes (p, A, B)
  outp  [XY, 24, Z*TS]       output, planes (s,p,c)

Blocks: 4 x (128 rows, z 0..24) + 1 x (64 rows, z halved across partitions).
Per (mu,sgn): proj h[j,p,b] -> products P[j,g,A,B] (h broadcast on outer A)
-> bsum over B (add1 on gpsimd, software-pipelined one term) -> combine
m[j,p,a] -> expand. Mass on ACT. Loads on SP queue, stores on ACT queue.
"""

import numpy as np

# ---------------------------------------------------------------- constants
X = Y = Z = 24
T = 48
NCORES = 8
TS = T // NCORES
TH = TS + 2
XY = X * Y
MASSP4 = 4.5

DIRSPEC = {
    0: dict(B=(3, 2), c=(-1j, -1j), e=(1, 0), d=(+1j, +1j)),
    1: dict(B=(3, 2), c=(-1, +1),   e=(1, 0), d=(+1, -1)),
    2: dict(B=(2, 3), c=(-1j, +1j), e=(0, 1), d=(+1j, -1j)),
    3: dict(B=(2, 3), c=(+1, +1),   e=(0, 1), d=(+1, +1)),
}

_CACHE = {}


def _split_waits_json(raw: bytes) -> bytes:
    """Walrus allows only ONE sync-wait per instruction: hoist extras onto
    NoOps inserted immediately before (same engine; sems monotonic => exact)."""
    import json
    bj = json.loads(raw)
    nid = 0
    for fn in bj.get("functions", []):
        for bb in fn.get("blocks", []):
            out = []
            changed = False
            for inst in bb.get("instructions", []):
                si = inst.get("sync_info")
                ow = (si or {}).get("on_wait") or []
                if len(ow) > 1:
                    changed = True
                    for w in ow[:-1]:
                        nid += 1
                        out.append({
                            "engine": inst["engine"], "ins": [], "outs": [],
                            "name": f"WSPL-{nid}", "opcode": "NoOp",
                            "sync_info": {"on_update": [], "on_wait": [w]},
                        })
                    si["on_wait"] = [ow[-1]]
                out.append(inst)
            if changed:
                bb["instructions"] = out
    return json.dumps(bj).encode()


def _install_json_wait_fix():
    import concourse.bass as bass
    if getattr(bass.Bass, "_wd_wait_fix", False):
        return
    orig = bass.Bass.to_json_bytes

    def patched(self, *a, **k):
        return _split_waits_json(orig(self, *a, **k))

    bass.Bass.to_json_bytes = patched
    bass.Bass._wd_wait_fix = True


def build_module(pool_add1=False):
    import concourse.bass as bass
    import concourse.mybir as mybir
    from concourse.ap import AP
    from concourse.mybir import AluOpType
    from concourse.tile import TileContext

    _install_json_wait_fix()
    F16 = mybir.dt.float16

    nc = bass.Bass()
    psi_h = nc.declare_dram_parameter("psi_h", [XY, 24, (Z + 2) * TH], F16, isOutput=False)
    fi4 = nc.declare_dram_parameter("fi4", [4, XY, 24, Z * TS], F16, isOutput=False)
    WFp = nc.declare_dram_parameter("WF", [4, XY, 18, Z * TS], F16, isOutput=False)
    WBp = nc.declare_dram_parameter("WB", [4, XY, 18, Z * TS], F16, isOutput=False)
    outp = nc.declare_dram_parameter("outp", [XY, 24, Z * TS], F16, isOutput=True)

    blocks = [(0, 128, [(0, 24, 0)]), (128, 128, [(0, 24, 0)]),
              (256, 128, [(0, 24, 0)]), (384, 128, [(0, 24, 0)]),
              (512, 64, [(0, 12, 0), (12, 12, 64)])]

    def sap(t, off, dims):
        return AP(t.tensor, t.offset + off, [list(t.ap[0])] + [list(d) for d in dims])

    with TileContext(nc) as tc:
        ctx_pool = tc.tile_pool(name="work", bufs=1)
        pool = ctx_pool.__enter__()
        V = nc.vector
        G = nc.gpsimd
        D = nc.sync
        ZFULL = Z * TS
        for (r0, R, parts) in blocks:
            zh = parts[0][1]
            npart = R * len(parts)
            ZT = zh * TS          # sites per row-slice
            PS = (zh + 2) * TH    # psi_al plane stride

            psi_al = pool.tile([npart, 24 * PS], F16, tag="psi_al", bufs=2)
            out_t = pool.tile([npart, 24 * ZT], F16, tag="out_t", bufs=2)
            # psi_h planar per-plane z-slice load (full-z: whole row)
            for (z0, _, p0) in parts:
                D.dma_start(out=psi_al[p0:p0 + R],
                            in_=psi_h[r0:r0 + R, :, z0 * TH:(z0 + zh + 2) * TH])

            def load(tag, src, mu, bufs, ncp):
                tl = pool.tile([npart, ncp * ZT], F16, tag=tag, bufs=bufs)
                for (z0, _, p0) in parts:
                    D.dma_start(out=tl[p0:p0 + R],
                                in_=src[mu, r0:r0 + R, :, z0 * TS:(z0 + zh) * TS])
                return tl

            # mass on ACT: out = 4.5 * psi interior (planes, z, t)
            nc.scalar.mul(
                sap(out_t, 0, [[ZT, 24], [6, zh], [1, 6]]),
                sap(psi_al, TH + 1, [[PS, 24], [TH, zh], [1, 6]]),
                MASSP4)

            tail_q = []

            EX = V
            def run_tail():
                if not tail_q:
                    return
                pt, st, mt, spec, dj = tail_q.pop()
                # bsum part 2 (add1 ran on gpsimd one term ago)
                V.tensor_tensor(sap(st, 0, [[ZT, 24], [1, ZT]]),
                                sap(st, 0, [[ZT, 24], [1, ZT]]),
                                sap(pt, 2 * ZT, [[3 * ZT, 24], [1, ZT]]),
                                AluOpType.add)
                # combine m[j,p,a]: m_re = S[g0]-S[g1]; m_im = S[g2]+S[g3]
                V.tensor_tensor(sap(mt, 0, [[6 * ZT, 2], [ZT, 3], [1, ZT]]),
                                sap(st, 0, [[12 * ZT, 2], [ZT, 3], [1, ZT]]),
                                sap(st, 3 * ZT, [[12 * ZT, 2], [ZT, 3], [1, ZT]]),
                                AluOpType.subtract)
                V.tensor_tensor(sap(mt, 3 * ZT, [[6 * ZT, 2], [ZT, 3], [1, ZT]]),
                                sap(st, 6 * ZT, [[12 * ZT, 2], [ZT, 3], [1, ZT]]),
                                sap(st, 9 * ZT, [[12 * ZT, 2], [ZT, 3], [1, ZT]]),
                                AluOpType.add)
                # expand
                d0, d1 = dj
                if d0.imag == 0.0 and d0.real == d1.real and spec["e"][0] == 0 \
                        and d0.real > 0:
                    # mu3 fwd: out[s] += m[s mod 2] for all 4 spins, one inst
                    EX.tensor_tensor(sap(out_t, 0, [[12 * ZT, 2], [ZT, 12], [1, ZT]]),
                                    sap(out_t, 0, [[12 * ZT, 2], [ZT, 12], [1, ZT]]),
                                    sap(mt, 0, [[0, 2], [ZT, 12], [1, ZT]]),
                                    AluOpType.add)
                    return
                o01 = sap(out_t, 0, [[ZT, 12], [1, ZT]])
                EX.tensor_tensor(o01, o01, sap(mt, 0, [[ZT, 12], [1, ZT]]),
                                AluOpType.add)
                if d0.imag == 0.0:
                    if d0.real == d1.real and spec["e"][0] == 0:
                        o23 = sap(out_t, 12 * ZT, [[ZT, 12], [1, ZT]])
                        EX.tensor_tensor(o23, o23, sap(mt, 0, [[ZT, 12], [1, ZT]]),
                                        AluOpType.add if d0.real > 0 else AluOpType.subtract)
                    else:
                        for si, (e, dv) in enumerate(zip(spec["e"], dj)):
                            os_ = sap(out_t, (12 + 6 * si) * ZT, [[ZT, 6], [1, ZT]])
                            EX.tensor_tensor(os_, os_, sap(mt, e * 6 * ZT, [[ZT, 6], [1, ZT]]),
                                            AluOpType.add if dv.real > 0 else AluOpType.subtract)
                else:
                    for si, (e, dv) in enumerate(zip(spec["e"], dj)):
                        sg = dv.imag > 0
                        ore = sap(out_t, (12 + 6 * si) * ZT, [[ZT, 3], [1, ZT]])
                        EX.tensor_tensor(ore, ore,
                                        sap(mt, (e * 6 + 3) * ZT, [[ZT, 3], [1, ZT]]),
                                        AluOpType.subtract if sg else AluOpType.add)
                        oim = sap(out_t, (12 + 6 * si + 3) * ZT, [[ZT, 3], [1, ZT]])
                        EX.tensor_tensor(oim, oim,
                                        sap(mt, e * 6 * ZT, [[ZT, 3], [1, ZT]]),
                                        AluOpType.add if sg else AluOpType.subtract)

            for mu in range(4):
                wf_t = load("w", WFp, mu, 6, 18)
                wb_t = load("w", WBp, mu, 6, 18)
                if mu <= 1:
                    pf_t = load("fi", fi4, 2 * mu, 8, 24)
                    pb_t = load("fi", fi4, 2 * mu + 1, 8, 24)
                spec = DIRSPEC[mu]

                for sgn in (+1, -1):
                    fwd = sgn > 0
                    cj = spec["c"] if fwd else tuple(-v for v in spec["c"])
                    dj = spec["d"] if fwd else tuple(-v for v in spec["d"])
                    wt = wf_t if fwd else wb_t

                    # psi source: planar planes; psi_al has (z,t) halo dims
                    if mu <= 1:
                        ps = pf_t if fwd else pb_t
                        pbase, pstr, pz = 0, ZT, [[1, ZT]]
                        hz = [[1, ZT]]
                    else:
                        if mu == 2:
                            pbase = (0 if fwd else 2 * TH) + 1
                        else:
                            pbase = TH + (0 if fwd else 2)
                        ps, pstr, pz = psi_al, PS, [[TH, zh], [1, 6]]
                        hz = [[6, zh], [1, 6]]

                    # --- proj h[j,p,b] = psi[A] + c*psi[B]  (plane-major)
                    ht = pool.tile([npart, 12 * ZT], F16, tag="h", bufs=2)
                    for j in (0, 1):
                        A, B, c = j, spec["B"][j], cj[j]
                        if c.imag == 0.0:
                            op = AluOpType.add if c.real > 0 else AluOpType.subtract
                            V.tensor_tensor(
                                sap(ht, j * 6 * ZT, [[ZT, 6]] + hz),
                                sap(ps, pbase + A * 6 * pstr, [[pstr, 6]] + pz),
                                sap(ps, pbase + B * 6 * pstr, [[pstr, 6]] + pz), op)
                        else:
                            sg = c.imag > 0
                            V.tensor_tensor(
                                sap(ht, j * 6 * ZT, [[ZT, 3]] + hz),
                                sap(ps, pbase + A * 6 * pstr, [[pstr, 3]] + pz),
                                sap(ps, pbase + (B * 6 + 3) * pstr, [[pstr, 3]] + pz),
                                AluOpType.subtract if sg else AluOpType.add)
                            V.tensor_tensor(
                                sap(ht, (j * 6 + 3) * ZT, [[ZT, 3]] + hz),
                                sap(ps, pbase + (A * 6 + 3) * pstr, [[pstr, 3]] + pz),
                                sap(ps, pbase + B * 6 * pstr, [[pstr, 3]] + pz),
                                AluOpType.add if sg else AluOpType.subtract)

                    # --- products P[j,g,A,B] planes = W[p,A,B] * h[j,p,B]
                    pt = pool.tile([npart, 72 * ZT], F16, tag="P", bufs=2)
                    for j in (0, 1):
                        for g, (wp, hp) in enumerate(((0, 0), (1, 1), (1, 0), (0, 1))):
                            V.tensor_tensor(
                                sap(pt, (j * 36 + g * 9) * ZT, [[3 * ZT, 3], [ZT, 3], [1, ZT]]),
                                sap(wt, wp * 9 * ZT, [[3 * ZT, 3], [ZT, 3], [1, ZT]]),
                                sap(ht, (j * 6 + hp * 3) * ZT, [[0, 3], [ZT, 3], [1, ZT]]),
                                AluOpType.mult)

                    # --- bsum part 1: S = P[B0] + P[B1]
                    st = pool.tile([npart, 24 * ZT], F16, tag="S", bufs=3)
                    E1 = G if pool_add1 else V
                    E1.tensor_tensor(sap(st, 0, [[ZT, 24], [1, ZT]]),
                                     sap(pt, 0, [[3 * ZT, 24], [1, ZT]]),
                                     sap(pt, ZT, [[3 * ZT, 24], [1, ZT]]),
                                     AluOpType.add)
                    mt = pool.tile([npart, 12 * ZT], F16, tag="m", bufs=6)

                    run_tail()
                    tail_q.append((pt, st, mt, spec, dj))

            run_tail()
            for (z0, _, p0) in parts:
                nc.scalar.dma_start(out=outp[r0:r0 + R, :, z0 * TS:(z0 + zh) * TS],
                                    in_=out_t[p0:p0 + R])
        ctx_pool.__exit__(None, None, None)
    return nc


# ---------------------------------------------------------------- host side
def prep_core_inputs(field, gauge, t0):
    """field [X,Y,Z,T,3,4] c64, gauge [4,X,Y,Z,T,3,3] c64 -> planar f16."""
    tsl = [(t0 + i) % T for i in range(TS)]
    th_idx = [(t0 - 1) % T] + tsl + [(t0 + TS) % T]
    f = field[:, :, :, th_idx]
    fr = np.stack([f.real, f.imag], axis=-1)            # [X,Y,Z,TH,c,s,p]
    fpl = fr.transpose(0, 1, 5, 6, 4, 2, 3)             # [X,Y,s,p,c,Z,TH]
    zhal = np.concatenate([fpl[..., -1:, :], fpl, fpl[..., :1, :]], axis=5)
    psi_h = np.ascontiguousarray(zhal).reshape(XY, 24 * (Z + 2) * TH).astype(np.float16)

    fin = fpl[..., :, 1:TS + 1]                         # [X,Y,s,p,c,Z,TS]
    rolls = [np.roll(fin, +1, 0), np.roll(fin, -1, 0),
             np.roll(fin, +1, 1), np.roll(fin, -1, 1)]
    fi4 = np.stack([np.ascontiguousarray(r).reshape(XY, 24 * Z * TS) for r in rolls]
                   ).astype(np.float16)

    WF = np.empty((4, XY, 18 * Z * TS), np.float16)
    WB = np.empty((4, XY, 18 * Z * TS), np.float16)
    for mu in range(4):
        Ub = gauge[mu][:, :, :, tsl]                    # [X,Y,Z,TS,A,B]
        vb = np.stack([Ub.real, Ub.imag], axis=4) * np.float32(-0.5)  # [X,Y,Z,TS,p,A,B]
        vbp = vb.transpose(0, 1, 4, 5, 6, 2, 3)         # [X,Y,p,A,B,Z,TS]
        WB[mu] = np.ascontiguousarray(vbp).reshape(XY, 18 * Z * TS).astype(np.float16)
        if mu == 3:
            tf = [(t0 - 1 + i) % T for i in range(TS)]
            Uf = gauge[mu][:, :, :, tf]
        else:
            Uf = np.roll(gauge[mu], +1, axis=mu)[:, :, :, tsl]
        Vf = np.conjugate(np.swapaxes(Uf, -1, -2))
        vf = np.stack([Vf.real, Vf.imag], axis=4) * np.float32(-0.5)
        vfp = vf.transpose(0, 1, 4, 5, 6, 2, 3)
        WF[mu] = np.ascontiguousarray(vfp).reshape(XY, 18 * Z * TS).astype(np.float16)
    return {"psi_h": psi_h, "fi4": fi4, "WF": WF, "WB": WB}


def prep_in_maps(field, gauge):
    return [prep_core_inputs(field, gauge, k * TS) for k in range(NCORES)]


def assemble_output(res):
    out = np.empty((X, Y, Z, T, 3, 4), np.complex64)
    for k in range(NCORES):
        o = res[k]["outp"].reshape(X, Y, 4, 2, 3, Z, TS).astype(np.float32)
        oc = (o[:, :, :, 0] + 1j * o[:, :, :, 1])       # [X,Y,s,c,Z,TS]
        out[:, :, :, k * TS:(k + 1) * TS] = oc.transpose(0, 1, 4, 5, 3, 2)
    return out


def kernel(field, gauge_field):
    from concourse.bass_utils import run_bass_kernel_spmd

    if "v3" not in _CACHE:
        _CACHE["v3"] = build_module()
    nc = _CACHE["v3"]
    in_maps = prep_in_maps(np.asarray(field), np.asarray(gauge_field))
    res = run_bass_kernel_spmd(nc, in_maps, list(range(NCORES))).results
    return assemble_output(res)


# revision 12
# speedup vs baseline: 3.0622x; 1.0973x over previous
"""Wilson-Dirac operator on Trainium2, 8 NeuronCores, T-axis domain decomposition.

v3: all-f16 compute in PLANAR (SoA) layout — every SBUF tile is
[component-plane][z*t] with the site dimension contiguous innermost, so all
hot DVE ops stream long 144-element runs (measured: short strided runs cost
~7-8 cycles each; planar removes them). DVE 2x packed f16 mode throughout.

Host arrays per core (f16, gauge pre-scaled by -0.5, fwd links pre-rolled,
all component-plane-major per lattice row):
  psi_h [XY, 24, Z+2, TS+2]  psi slab, z+t halos, planes (s,p,c)
  fi4   [4, XY, 24, Z*TS]    interior psi pre-rolled (x+1, x-1, y+1, y-1)
  WF/WB [4, XY, 18, Z*TS]    hopping matrices, planes (p, A, B)
  outp  [XY, 24, Z*TS]       output, planes (s,p,c)

Blocks: 4 x (128 rows, z 0..24) + 1 x (64 rows, z halved across partitions).
Per (mu,sgn): proj h[j,p,b] -> products P[j,g,A,B] (h broadcast on outer A)
-> bsum over B (add1 on gpsimd, software-pipelined one term) -> combine
m[j,p,a] -> expand. Mass on ACT. Loads on SP queue, stores on ACT queue.
"""

import numpy as np

# ---------------------------------------------------------------- constants
X = Y = Z = 24
T = 48
NCORES = 8
TS = T // NCORES
TH = TS + 2
XY = X * Y
MASSP4 = 4.5

DIRSPEC = {
    0: dict(B=(3, 2), c=(-1j, -1j), e=(1, 0), d=(+1j, +1j)),
    1: dict(B=(3, 2), c=(-1, +1),   e=(1, 0), d=(+1, -1)),
    2: dict(B=(2, 3), c=(-1j, +1j), e=(0, 1), d=(+1j, -1j)),
    3: dict(B=(2, 3), c=(+1, +1),   e=(0, 1), d=(+1, +1)),
}

_CACHE = {}


def _split_waits_json(raw: bytes) -> bytes:
    """Walrus allows only ONE sync-wait per instruction: hoist extras onto
    NoOps inserted immediately before (same engine; sems monotonic => exact)."""
    import json
    bj = json.loads(raw)
    nid = 0
    for fn in bj.get("functions", []):
        for bb in fn.get("blocks", []):
            out = []
            changed = False
            for inst in bb.get("instructions", []):
                si = inst.get("sync_info")
                ow = (si or {}).get("on_wait") or []
                if len(ow) > 1:
                    changed = True
                    for w in ow[:-1]:
                        nid += 1
                        out.append({
                            "engine": inst["engine"], "ins": [], "outs": [],
                            "name": f"WSPL-{nid}", "opcode": "NoOp",
                            "sync_info": {"on_update": [], "on_wait": [w]},
                        })
                    si["on_wait"] = [ow[-1]]
                out.append(inst)
            if changed:
                bb["instructions"] = out
    return json.dumps(bj).encode()


def _install_json_wait_fix():
    import concourse.bass as bass
    if getattr(bass.Bass, "_wd_wait_fix", False):
        return
    orig = bass.Bass.to_json_bytes

    def patched(self, *a, **k):
        return _split_waits_json(orig(self, *a, **k))

    bass.Bass.to_json_bytes = patched
    bass.Bass._wd_wait_fix = True


def build_module(pool_add1=False):
    import concourse.bass as bass
    import concourse.mybir as mybir
    from concourse.ap import AP
    from concourse.mybir import AluOpType
    from concourse.tile import TileContext

    _install_json_wait_fix()
    F16 = mybir.dt.float16

    nc = bass.Bass()
    psi_h = nc.declare_dram_parameter("psi_h", [XY, 24, (Z + 2) * TH], F16, isOutput=False)
    fi4 = nc.declare_dram_parameter("fi4", [4, XY, 24, Z * TS], F16, isOutput=False)
    WFp = nc.declare_dram_parameter("WF", [4, XY, 27, Z * TS], F16, isOutput=False)
    WBp = nc.declare_dram_parameter("WB", [4, XY, 27, Z * TS], F16, isOutput=False)
    outp = nc.declare_dram_parameter("outp", [XY, 24, Z * TS], F16, isOutput=True)

    blocks = [(0, 128, [(0, 24, 0)]), (128, 128, [(0, 24, 0)]),
              (256, 128, [(0, 24, 0)]), (384, 128, [(0, 24, 0)]),
              (512, 64, [(0, 12, 0), (12, 12, 64)])]

    def sap(t, off, dims):
        return AP(t.tensor, t.offset + off, [list(t.ap[0])] + [list(d) for d in dims])

    with TileContext(nc) as tc:
        ctx_pool = tc.tile_pool(name="work", bufs=1)
        pool = ctx_pool.__enter__()
        V = nc.vector
        G = nc.gpsimd
        D = nc.sync
        ZFULL = Z * TS
        for (r0, R, parts) in blocks:
            zh = parts[0][1]
            npart = R * len(parts)
            ZT = zh * TS          # sites per row-slice
            PS = (zh + 2) * TH    # psi_al plane stride

            psi_al = pool.tile([npart, 24 * PS], F16, tag="psi_al", bufs=3)
            out_t = pool.tile([npart, 24 * ZT], F16, tag="out_t", bufs=2)
            # psi_h planar per-plane z-slice load (full-z: whole row)
            for (z0, _, p0) in parts:
                D.dma_start(out=psi_al[p0:p0 + R],
                            in_=psi_h[r0:r0 + R, :, z0 * TH:(z0 + zh + 2) * TH])

            def load(tag, src, mu, bufs, ncp):
                tl = pool.tile([npart, ncp * ZT], F16, tag=tag, bufs=bufs)
                for (z0, _, p0) in parts:
                    D.dma_start(out=tl[p0:p0 + R],
                                in_=src[mu, r0:r0 + R, :, z0 * TS:(z0 + zh) * TS])
                return tl

            # mass on ACT: out = 4.5 * psi interior (planes, z, t)
            nc.scalar.mul(
                sap(out_t, 0, [[ZT, 24], [6, zh], [1, 6]]),
                sap(psi_al, TH + 1, [[PS, 24], [TH, zh], [1, 6]]),
                MASSP4)

            tail_q = []

            EX = V
            def run_tail():
                if not tail_q:
                    return
                pt, st, mt, spec, dj = tail_q.pop()
                # bsum part 2 (add1 ran on gpsimd one term ago)
                V.tensor_tensor(sap(st, 0, [[ZT, 18], [1, ZT]]),
                                sap(st, 0, [[ZT, 18], [1, ZT]]),
                                sap(pt, 2 * ZT, [[3 * ZT, 18], [1, ZT]]),
                                AluOpType.add)
                # combine: m_re = S1-S2; tmp(m_im) = S1+S2; m_im = S5-tmp
                V.tensor_tensor(sap(mt, 0, [[6 * ZT, 2], [ZT, 3], [1, ZT]]),
                                sap(st, 0, [[9 * ZT, 2], [ZT, 3], [1, ZT]]),
                                sap(st, 3 * ZT, [[9 * ZT, 2], [ZT, 3], [1, ZT]]),
                                AluOpType.subtract)
                V.tensor_tensor(sap(mt, 3 * ZT, [[6 * ZT, 2], [ZT, 3], [1, ZT]]),
                                sap(st, 0, [[9 * ZT, 2], [ZT, 3], [1, ZT]]),
                                sap(st, 3 * ZT, [[9 * ZT, 2], [ZT, 3], [1, ZT]]),
                                AluOpType.add)
                V.tensor_tensor(sap(mt, 3 * ZT, [[6 * ZT, 2], [ZT, 3], [1, ZT]]),
                                sap(st, 6 * ZT, [[9 * ZT, 2], [ZT, 3], [1, ZT]]),
                                sap(mt, 3 * ZT, [[6 * ZT, 2], [ZT, 3], [1, ZT]]),
                                AluOpType.subtract)
                # expand
                d0, d1 = dj
                if d0.imag == 0.0 and d0.real == d1.real and spec["e"][0] == 0 \
                        and d0.real > 0:
                    # mu3 fwd: out[s] += m[s mod 2] for all 4 spins, one inst
                    EX.tensor_tensor(sap(out_t, 0, [[12 * ZT, 2], [ZT, 12], [1, ZT]]),
                                    sap(out_t, 0, [[12 * ZT, 2], [ZT, 12], [1, ZT]]),
                                    sap(mt, 0, [[0, 2], [ZT, 12], [1, ZT]]),
                                    AluOpType.add)
                    return
                o01 = sap(out_t, 0, [[ZT, 12], [1, ZT]])
                EX.tensor_tensor(o01, o01, sap(mt, 0, [[ZT, 12], [1, ZT]]),
                                AluOpType.add)
                if d0.imag == 0.0:
                    if d0.real == d1.real and spec["e"][0] == 0:
                        o23 = sap(out_t, 12 * ZT, [[ZT, 12], [1, ZT]])
                        EX.tensor_tensor(o23, o23, sap(mt, 0, [[ZT, 12], [1, ZT]]),
                                        AluOpType.add if d0.real > 0 else AluOpType.subtract)
                    else:
                        for si, (e, dv) in enumerate(zip(spec["e"], dj)):
                            os_ = sap(out_t, (12 + 6 * si) * ZT, [[ZT, 6], [1, ZT]])
                            EX.tensor_tensor(os_, os_, sap(mt, e * 6 * ZT, [[ZT, 6], [1, ZT]]),
                                            AluOpType.add if dv.real > 0 else AluOpType.subtract)
                else:
                    for si, (e, dv) in enumerate(zip(spec["e"], dj)):
                        sg = dv.imag > 0
                        ore = sap(out_t, (12 + 6 * si) * ZT, [[ZT, 3], [1, ZT]])
                        EX.tensor_tensor(ore, ore,
                                        sap(mt, (e * 6 + 3) * ZT, [[ZT, 3], [1, ZT]]),
                                        AluOpType.subtract if sg else AluOpType.add)
                        oim = sap(out_t, (12 + 6 * si + 3) * ZT, [[ZT, 3], [1, ZT]])
                        EX.tensor_tensor(oim, oim,
                                        sap(mt, e * 6 * ZT, [[ZT, 3], [1, ZT]]),
                                        AluOpType.add if sg else AluOpType.subtract)

            for mu in range(4):
                wf_t = load("w", WFp, mu, 6, 27)
                wb_t = load("w", WBp, mu, 6, 27)
                if mu <= 1:
                    pf_t = load("fi", fi4, 2 * mu, 6, 24)
                    pb_t = load("fi", fi4, 2 * mu + 1, 6, 24)
                spec = DIRSPEC[mu]

                for sgn in (+1, -1):
                    fwd = sgn > 0
                    cj = spec["c"] if fwd else tuple(-v for v in spec["c"])
                    dj = spec["d"] if fwd else tuple(-v for v in spec["d"])
                    wt = wf_t if fwd else wb_t

                    # psi source: planar planes; psi_al has (z,t) halo dims
                    if mu <= 1:
                        ps = pf_t if fwd else pb_t
                        pbase, pstr, pz = 0, ZT, [[1, ZT]]
                        hz = [[1, ZT]]
                    else:
                        if mu == 2:
                            pbase = (0 if fwd else 2 * TH) + 1
                        else:
                            pbase = TH + (0 if fwd else 2)
                        ps, pstr, pz = psi_al, PS, [[TH, zh], [1, 6]]
                        hz = [[6, zh], [1, 6]]

                    # --- proj h[j,p,b] = psi[A] + c*psi[B]  (plane-major)
                    ht = pool.tile([npart, 18 * ZT], F16, tag="h", bufs=2)
                    for j in (0, 1):
                        A, B, c = j, spec["B"][j], cj[j]
                        if c.imag == 0.0:
                            op = AluOpType.add if c.real > 0 else AluOpType.subtract
                            V.tensor_tensor(
                                sap(ht, j * 9 * ZT, [[ZT, 6]] + hz),
                                sap(ps, pbase + A * 6 * pstr, [[pstr, 6]] + pz),
                                sap(ps, pbase + B * 6 * pstr, [[pstr, 6]] + pz), op)
                        else:
                            sg = c.imag > 0
                            V.tensor_tensor(
                                sap(ht, j * 9 * ZT, [[ZT, 3]] + hz),
                                sap(ps, pbase + A * 6 * pstr, [[pstr, 3]] + pz),
                                sap(ps, pbase + (B * 6 + 3) * pstr, [[pstr, 3]] + pz),
                                AluOpType.subtract if sg else AluOpType.add)
                            V.tensor_tensor(
                                sap(ht, (j * 9 + 3) * ZT, [[ZT, 3]] + hz),
                                sap(ps, pbase + (A * 6 + 3) * pstr, [[pstr, 3]] + pz),
                                sap(ps, pbase + B * 6 * pstr, [[pstr, 3]] + pz),
                                AluOpType.add if sg else AluOpType.subtract)

                    # --- hsum: h[j,sum,b] = h[j,re,b] + h[j,im,b]
                    V.tensor_tensor(sap(ht, 6 * ZT, [[9 * ZT, 2], [ZT, 3], [1, ZT]]),
                                    sap(ht, 0, [[9 * ZT, 2], [ZT, 3], [1, ZT]]),
                                    sap(ht, 3 * ZT, [[9 * ZT, 2], [ZT, 3], [1, ZT]]),
                                    AluOpType.add)
                    # --- Karatsuba products P[j,k,A,B] = W[k,A,B] * h[j,k,B]
                    # k: (re*hre, im*him, sum*hsum)
                    pt = pool.tile([npart, 54 * ZT], F16, tag="P", bufs=2)
                    for j in (0, 1):
                        for k in range(3):
                            V.tensor_tensor(
                                sap(pt, (j * 27 + k * 9) * ZT, [[3 * ZT, 3], [ZT, 3], [1, ZT]]),
                                sap(wt, k * 9 * ZT, [[3 * ZT, 3], [ZT, 3], [1, ZT]]),
                                sap(ht, (j * 9 + k * 3) * ZT, [[0, 3], [ZT, 3], [1, ZT]]),
                                AluOpType.mult)

                    # --- bsum part 1: S = P[B0] + P[B1]
                    st = pool.tile([npart, 18 * ZT], F16, tag="S", bufs=3)
                    V.tensor_tensor(sap(st, 0, [[ZT, 18], [1, ZT]]),
                                    sap(pt, 0, [[3 * ZT, 18], [1, ZT]]),
                                    sap(pt, ZT, [[3 * ZT, 18], [1, ZT]]),
                                    AluOpType.add)
                    mt = pool.tile([npart, 12 * ZT], F16, tag="m", bufs=3)

                    run_tail()
                    tail_q.append((pt, st, mt, spec, dj))

            run_tail()
            for (z0, _, p0) in parts:
                nc.scalar.dma_start(out=outp[r0:r0 + R, :, z0 * TS:(z0 + zh) * TS],
                                    in_=out_t[p0:p0 + R])
        ctx_pool.__exit__(None, None, None)
    return nc


# ---------------------------------------------------------------- host side
def prep_core_inputs(field, gauge, t0):
    """field [X,Y,Z,T,3,4] c64, gauge [4,X,Y,Z,T,3,3] c64 -> planar f16."""
    tsl = [(t0 + i) % T for i in range(TS)]
    th_idx = [(t0 - 1) % T] + tsl + [(t0 + TS) % T]
    f = field[:, :, :, th_idx]
    fr = np.stack([f.real, f.imag], axis=-1)            # [X,Y,Z,TH,c,s,p]
    fpl = fr.transpose(0, 1, 5, 6, 4, 2, 3)             # [X,Y,s,p,c,Z,TH]
    zhal = np.concatenate([fpl[..., -1:, :], fpl, fpl[..., :1, :]], axis=5)
    psi_h = np.ascontiguousarray(zhal).reshape(XY, 24 * (Z + 2) * TH).astype(np.float16)

    fin = fpl[..., :, 1:TS + 1]                         # [X,Y,s,p,c,Z,TS]
    rolls = [np.roll(fin, +1, 0), np.roll(fin, -1, 0),
             np.roll(fin, +1, 1), np.roll(fin, -1, 1)]
    fi4 = np.stack([np.ascontiguousarray(r).reshape(XY, 24 * Z * TS) for r in rolls]
                   ).astype(np.float16)

    WF = np.empty((4, XY, 27 * Z * TS), np.float16)
    WB = np.empty((4, XY, 27 * Z * TS), np.float16)
    for mu in range(4):
        Ub = gauge[mu][:, :, :, tsl]                    # [X,Y,Z,TS,A,B]
        vb = np.stack([Ub.real, Ub.imag, Ub.real + Ub.imag], axis=4) * np.float32(-0.5)
        vbp = vb.transpose(0, 1, 4, 5, 6, 2, 3)         # [X,Y,k,A,B,Z,TS]
        WB[mu] = np.ascontiguousarray(vbp).reshape(XY, 27 * Z * TS).astype(np.float16)
        if mu == 3:
            tf = [(t0 - 1 + i) % T for i in range(TS)]
            Uf = gauge[mu][:, :, :, tf]
        else:
            Uf = np.roll(gauge[mu], +1, axis=mu)[:, :, :, tsl]
        Vf = np.conjugate(np.swapaxes(Uf, -1, -2))
        vf = np.stack([Vf.real, Vf.imag, Vf.real + Vf.imag], axis=4) * np.float32(-0.5)
        vfp = vf.transpose(0, 1, 4, 5, 6, 2, 3)
        WF[mu] = np.ascontiguousarray(vfp).reshape(XY, 27 * Z * TS).astype(np.float16)
    return {"psi_h": psi_h, "fi4": fi4, "WF": WF, "WB": WB}


def prep_in_maps(field, gauge):
    return [prep_core_inputs(field, gauge, k * TS) for k in range(NCORES)]


def assemble_output(res):
    out = np.empty((X, Y, Z, T, 3, 4), np.complex64)
    for k in range(NCORES):
        o = res[k]["outp"].reshape(X, Y, 4, 2, 3, Z, TS).astype(np.float32)
        oc = (o[:, :, :, 0] + 1j * o[:, :, :, 1])       # [X,Y,s,c,Z,TS]
        out[:, :, :, k * TS:(k + 1) * TS] = oc.transpose(0, 1, 4, 5, 3, 2)
    return out


def kernel(field, gauge_field):
    from concourse.bass_utils import run_bass_kernel_spmd

    if "v3" not in _CACHE:
        _CACHE["v3"] = build_module()
    nc = _CACHE["v3"]
    in_maps = prep_in_maps(np.asarray(field), np.asarray(gauge_field))
    res = run_bass_kernel_spmd(nc, in_maps, list(range(NCORES))).results
    return assemble_output(res)
